# revision 12
# baseline (speedup 1.0000x reference)
"""GCN (GraphConv norm='both' -> ReLU -> SAGEConv mean) on 8 Trainium2 NeuronCores.

Contract: kernel(**inputs) takes the FULL inputs from setup_inputs() and
returns the FULL [N, OUT] output.

Sharding strategy (graph/data parallel, per the problem's sharding hint):
  - Nodes are partitioned contiguously across the 8 cores (12500 each).
  - Edges are partitioned by the owner of their *dst* node; each core's
    edges are bucketed per 128-node dst window into padded 128-edge chunks
    and aggregated with one-hot matmuls on the TensorEngine.
  - All gathered/streamed feature data is bf16 (tolerance is 2e-2; the
    bf16 pipeline sims at ~4e-3), halving the dominant cost: the random
    256-byte-per-edge dma_gather traffic, and running the one-hot matmuls
    at full PE rate (fp32 matmul is 4 cyc/row, bf16 is 1).
  - The degree normalization is split so the one-hot matrices stay pure
    0/1 (one DVE is_equal per group instead of two ops): s_out[src] is
    folded into x on the host (xb = s_out*x in bf16), and s_in[dst] is a
    per-dst-column scale fused into the PSUM->SBUF copy before W1.
  - Phase 1 (per core): dma_gather xb rows (4 int16-index ranges),
    one-hot matmul segment-sum into PSUM, hT = relu(W1.T@(agg*s_in)+b1)
    kept SBUF-resident in bf16, z = h @ W_neigh written (bf16) to a local
    z shard viewed as row-pairs.
  - Halo exchange: z (bf16, 64 wide = 12.8 MB total) is AllGathered in
    TWO halves so the first collective overlaps the second half of
    phase 1.  Nodes are remapped (host-side) so each half is rank-major
    contiguous.
  - Phase 2 (per core): dma_gather z-row-PAIRS (256B descriptors -- the
    gather elem must be a multiple of 256B, so single 128B bf16 z rows
    cannot be gathered directly).  Host packs edges into parity-pure
    chunks so each chunk's matmul reads the correct 64-column half of the
    gathered pair.  Segment-sum with 0/1 one-hots, scale by 1/deg_in,
    add h @ W_self + b2, write the core's [12500, 64] fp32 output shard.
  - Host concatenates the 8 shards.

Engine assignment is chosen to avoid FIFO head-of-line serialization:
the full int16 gather-index arrays are preloaded into SBUF (no per-group
index DMAs on the Sync engine), all PSUM->SBUF copies run on the Scalar
engine, and the Vector engine runs only the one-hot builds (software-
pipelined one group ahead) plus small per-window scales.
"""

import os
import sys
from contextlib import ExitStack

import numpy as np
import ml_dtypes

BF = ml_dtypes.bfloat16

for _p in ("/opt/trn_rl_repo", "/opt/pypackages"):
    if _p not in sys.path:
        sys.path.append(_p)

import concourse.bacc as bacc
import concourse.bass as bass
import concourse.mybir as mybir
import concourse.tile as tile
from concourse.bass_utils import run_bass_kernel_spmd

F32 = mybir.dt.float32
BF16 = mybir.dt.bfloat16
I16 = mybir.dt.int16
AOT = mybir.AluOpType
AFT = mybir.ActivationFunctionType

N_CORES = 8
WIN = 128
RSZ1 = 32768          # phase-1 src index range (int16 limit)
GROUP = 4             # windows per gather/eq group
SUBCHUNKS = int(os.environ.get("GCN_SUB", "64"))   # max chunks per dma_gather
NQUEUES = 4
HALF_W = 49           # windows per allgather half (49*128 = 6272 rows)
GATHER_BUFS = int(os.environ.get("GCN_GB", "4"))
SCRATCH = int(os.environ.get("GCN_SCRATCH", "16384"))  # SWDGE desc carveout B/partition


def _install_ntff_hook_shim():
    """The agent image's antenv lacks axon_hooks; provide it so trace=True
    can capture NTFF profiles through libaxon."""
    try:
        from antenv import axon_hooks  # noqa: F401
        return
    except ImportError:
        pass
    try:
        import types

        import antenv
        from trn_agent_boot.trn_boot import _ntff_profile_via_ctypes

        mod = types.ModuleType("antenv.axon_hooks")
        mod._hook = _ntff_profile_via_ctypes("/opt/axon/libaxon_pjrt.so")

        def get_axon_ntff_profile_hook():
            return mod._hook

        def set_axon_ntff_profile_hook(h):
            mod._hook = h

        mod.get_axon_ntff_profile_hook = get_axon_ntff_profile_hook
        mod.set_axon_ntff_profile_hook = set_axon_ntff_profile_hook
        sys.modules["antenv.axon_hooks"] = mod
        antenv.axon_hooks = mod
    except Exception:
        pass


_install_ntff_hook_shim()


# ---------------------------------------------------------------------------
# Host-side graph prep
# ---------------------------------------------------------------------------

class Phase:
    """Chunked edge-bucket structure for one gather/segment-sum phase."""
    pass


def _build_phase(owner, wrow, code, idx_local, cell_of, ncells, range_of_cell,
                 n_cores, NW, sub):
    """Bucket edges into per-(window, cell) 128-slot chunks, padded to the
    max count over cores so the SPMD program is identical on all cores.

    cell_of: per-edge cell id in [0, ncells); range_of_cell: gather source
    range per cell (cells sharing a range share a gather slab).
    """
    nranges = int(max(range_of_cell)) + 1
    counts = np.zeros((n_cores, NW, ncells), np.int64)
    np.add.at(counts, (owner, wrow, cell_of), 1)
    cwr = (counts.max(axis=0) + WIN - 1) // WIN          # [NW, ncells]
    empty = cwr.sum(axis=1) == 0
    cwr[empty, 0] = 1
    n_w = cwr.sum(axis=1)

    groups = [(g0, min(g0 + GROUP, NW)) for g0 in range(0, NW, GROUP)]

    # gather order: group -> range -> window -> cell(in range) -> chunk
    cell_start = np.zeros((NW, ncells), np.int64)
    slab_meta = []                                       # [g][r] = (start, n)
    c = 0
    for g0, g1 in groups:
        metas = []
        for r in range(nranges):
            s = c
            for w in range(g0, g1):
                for cl in range(ncells):
                    if range_of_cell[cl] != r:
                        continue
                    cell_start[w, cl] = c
                    c += int(cwr[w, cl])
            metas.append((s, c - s))
        slab_meta.append(metas)
    C = c

    gathers = []                # (g, r, chunk_off_in_slab, nb, global_chunk)
    if os.environ.get("GCN_CELLGATHER", "0") == "1":
        # one gather per (window, cell): pad slots carry trailing -1 indices,
        # which the SWDGE gather ucode skips (no descriptors generated)
        for g, (g0, g1) in enumerate(groups):
            for r in range(nranges):
                s, n = slab_meta[g][r]
                for w in range(g0, g1):
                    for cl in range(ncells):
                        if range_of_cell[cl] != r:
                            continue
                        cs = int(cell_start[w, cl])
                        nb = int(cwr[w, cl])
                        for i in range(0, nb, sub):
                            nbb = min(sub, nb - i)
                            gathers.append((g, r, cs - s + i, nbb, cs + i))
    else:
        for g in range(len(groups)):
            for r in range(nranges):
                s, n = slab_meta[g][r]
                for i in range(0, n, sub):
                    nb = min(sub, n - i)
                    gathers.append((g, r, i, nb, s + i))

    # window-major chunk columns (for the one-hot code arrays)
    wc0 = np.zeros(NW, np.int64)
    wc0[1:] = np.cumsum(n_w)[:-1]

    window_chunks = []          # [w] -> list of (cell, gather_chunk_id)
    for w in range(NW):
        lst = []
        for cl in range(ncells):
            for j in range(int(cwr[w, cl])):
                lst.append((cl, int(cell_start[w, cl]) + j))
        window_chunks.append(lst)

    per_core = []
    for k in range(n_cores):
        m = owner == k
        key = (wrow[m] * ncells + cell_of[m]).astype(np.int64)
        order = np.argsort(key, kind="stable")
        key = key[order]
        e_idx = idx_local[m][order]
        e_code = code[m][order]
        bounds = np.searchsorted(key, np.arange(NW * ncells + 1))

        cellgather = os.environ.get("GCN_CELLGATHER", "0") == "1"
        A_idx = np.zeros(C * WIN, np.int16)
        W_code = np.full(C * WIN, 255.0, np.float32)
        for w in range(NW):
            woff = 0
            for cl in range(ncells):
                a, b = bounds[w * ncells + cl], bounds[w * ncells + cl + 1]
                n = b - a
                gbase = int(cell_start[w, cl]) * WIN
                wbase = (int(wc0[w]) + woff) * WIN
                woff += int(cwr[w, cl])
                if n > 0:
                    A_idx[gbase : gbase + n] = e_idx[a:b].astype(np.int16)
                    W_code[wbase : wbase + n] = e_code[a:b]
                if cellgather:
                    # trailing pads generate no DMA descriptors; keep >=128
                    # valid slots (a full partition sweep) so every SDMA
                    # engine gets a descriptor and increments the sem
                    slots = int(cwr[w, cl]) * WIN
                    A_idx[gbase + max(n, 128) : gbase + slots] = -1

        eidx = np.ascontiguousarray(np.tile(A_idx.reshape(C * 8, 16).T, (8, 1)))
        ecode = np.ascontiguousarray(W_code.reshape(C, WIN).T)
        per_core.append((eidx, ecode))

    ph = Phase()
    ph.nranges = nranges
    ph.cwr = cwr
    ph.n_w = n_w
    ph.wc0 = wc0
    ph.C = C
    ph.groups = groups
    ph.slab_meta = slab_meta
    ph.gathers = gathers
    ph.window_chunks = window_chunks
    ph.per_core = per_core
    ph.max_nw = int(n_w.max())
    ph.group_nw = [int(n_w[g0:g1].sum()) for g0, g1 in groups]
    ph.max_group_nw = max(ph.group_nw)
    ph.group_c0 = [slab_meta[g][0][0] for g in range(len(groups))]
    ph.group_c1 = ph.group_c0[1:] + [C]
    ph.max_slab = [
        max((slab_meta[g][r][1] for g in range(len(groups))), default=0)
        for r in range(nranges)
    ]
    by_slab = {}
    for gi, (g, r, i, nb, cs) in enumerate(gathers):
        by_slab.setdefault((g, r), []).append((gi, i, nb, cs))
    ph.by_slab = by_slab
    return ph


class Prep:
    pass


def prepare(src, dst, n_nodes, n_cores=N_CORES):
    src = np.asarray(src).astype(np.int64)
    dst = np.asarray(dst).astype(np.int64)
    P = n_nodes // n_cores
    assert P * n_cores == n_nodes
    NW = (P + WIN - 1) // WIN
    rows_last = P - WIN * (NW - 1)
    rows_a = HALF_W * WIN                # 6272 local rows in allgather half A
    rows_b = P - rows_a
    assert rows_a % 2 == 0 and rows_b % 2 == 0

    deg_out = np.bincount(src, minlength=n_nodes).astype(np.float32)
    deg_in = np.bincount(dst, minlength=n_nodes).astype(np.float32)
    s_out = np.where(deg_out > 0, 1.0 / np.sqrt(np.maximum(deg_out, 1.0)), 0.0)
    s_in = np.where(deg_in > 0, 1.0 / np.sqrt(np.maximum(deg_in, 1.0)), 0.0)
    invd = (1.0 / np.maximum(deg_in, 1.0)).astype(np.float32)

    owner = dst // P
    ldst = dst - owner * P
    wrow = ldst // WIN
    code = (ldst % WIN).astype(np.float32)

    # ---- phase 1: gather xb rows; cells = 4 int16 src ranges ----
    rng1 = np.minimum(src // RSZ1, 3)
    idx1 = src - rng1 * RSZ1
    ph1 = _build_phase(owner, wrow, code, idx1, rng1, 4, [0, 1, 2, 3],
                       n_cores, NW, SUBCHUNKS)

    # ---- phase 2: gather z row-pairs from the remapped (split-allgather)
    # z layout; cells = (pair-range A/B) x (row parity) ----
    sc = src // P
    sl = src - sc * P
    new_row = np.where(sl < rows_a,
                       sc * rows_a + sl,
                       n_cores * rows_a + sc * rows_b + (sl - rows_a))
    pairs_a = n_cores * rows_a // 2      # 25088
    pr = new_row >> 1
    parity = (new_row & 1).astype(np.int64)
    r2 = (pr >= pairs_a).astype(np.int64)
    idx2 = pr - r2 * pairs_a
    cell2 = r2 * 2 + parity
    ph2 = _build_phase(owner, wrow, code, idx2, cell2, 4, [0, 0, 1, 1],
                       n_cores, NW, SUBCHUNKS)

    per_core = []
    for k in range(n_cores):
        eidx1, ecode1 = ph1.per_core[k]
        eidx2, ecode2 = ph2.per_core[k]
        nodes = np.arange(P) + k * P
        iv = np.zeros(NW * WIN, np.float32)
        iv[:P] = invd[nodes]
        sr = np.zeros(NW * WIN, np.float32)
        sr[:P] = s_in[nodes]
        per_core.append(dict(
            eidx1=eidx1, ecode1=ecode1,
            eidx2=eidx2, ecode2=ecode2,
            invd=np.ascontiguousarray(iv.reshape(NW, WIN).T),
            sr=np.ascontiguousarray(np.broadcast_to(sr, (WIN, NW * WIN))),
        ))

    p = Prep()
    p.P, p.NW, p.rows_last = P, NW, rows_last
    p.rows_a, p.rows_b = rows_a, rows_b
    p.pairs_a = pairs_a
    p.pairs_b = (n_nodes - n_cores * rows_a) // 2
    p.ph1, p.ph2 = ph1, ph2
    p.per_core = per_core
    p.s_out = s_out
    p.n_nodes = n_nodes
    p.n_cores = n_cores
    return p


# ---------------------------------------------------------------------------
# Bass/Tile kernel builder
# ---------------------------------------------------------------------------

def build_gcn(p, F, H, O):
    NW, P = p.NW, p.P
    ph1, ph2 = p.ph1, p.ph2
    groups = ph1.groups
    ngroups = len(groups)

    nc = bacc.Bacc(
        "TRN2", debug=False, enable_asserts=False, num_devices=p.n_cores,
        num_swdge_queues=NQUEUES, dynamic_dma_scratch_size=SCRATCH,
    )

    x_d = nc.dram_tensor("x", [p.n_nodes, F], BF16, kind="ExternalInput").ap()
    W1_d = nc.dram_tensor("W1", [F, H], BF16, kind="ExternalInput").ap()
    b1_d = nc.dram_tensor("b1", [H, 1], F32, kind="ExternalInput").ap()
    Ws_d = nc.dram_tensor("W_self", [H, O], BF16, kind="ExternalInput").ap()
    Wn_d = nc.dram_tensor("W_neigh", [H, O], BF16, kind="ExternalInput").ap()
    b2_d = nc.dram_tensor("b2", [1, O], BF16, kind="ExternalInput").ap()
    eidx1_d = nc.dram_tensor("eidx1", [WIN, ph1.C * 8], I16, kind="ExternalInput").ap()
    ecode1_d = nc.dram_tensor("ecode1", [WIN, ph1.C], F32, kind="ExternalInput").ap()
    eidx2_d = nc.dram_tensor("eidx2", [WIN, ph2.C * 8], I16, kind="ExternalInput").ap()
    ecode2_d = nc.dram_tensor("ecode2", [WIN, ph2.C], F32, kind="ExternalInput").ap()
    invd_d = nc.dram_tensor("invd", [WIN, NW], F32, kind="ExternalInput").ap()
    sr_d = nc.dram_tensor("sr", [WIN, NW * WIN], F32, kind="ExternalInput").ap()
    out_d = nc.dram_tensor("out", [P, O], F32, kind="ExternalOutput").ap()

    qn = [0]

    def next_q():
        q = qn[0]
        qn[0] = (q + 1) % NQUEUES
        return q

    with tile.TileContext(nc, num_cores=p.n_cores) as tc, ExitStack() as ctx:
        const = ctx.enter_context(tc.tile_pool(name="const", bufs=1))
        dram = ctx.enter_context(tc.tile_pool(name="dram", bufs=1, space="DRAM"))

        W1s = const.tile([F, H], BF16)
        nc.sync.dma_start(W1s[:], W1_d)
        Wss = const.tile([H, O], BF16)
        nc.sync.dma_start(Wss[:], Ws_d)
        Wns = const.tile([H, O], BF16)
        nc.sync.dma_start(Wns[:], Wn_d)
        b1s = const.tile([H, 1], F32)
        nc.sync.dma_start(b1s[:], b1_d)
        b2s = const.tile([1, O], BF16)
        nc.sync.dma_start(b2s[:], b2_d)
        ecode1_s = const.tile([WIN, ph1.C], F32)
        nc.sync.dma_start(ecode1_s[:], ecode1_d)
        ecode2_s = const.tile([WIN, ph2.C], F32)
        nc.sync.dma_start(ecode2_s[:], ecode2_d)
        invd_s = const.tile([WIN, NW], F32)
        nc.sync.dma_start(invd_s[:], invd_d)

        ones1 = const.tile([1, WIN], BF16)
        nc.vector.memset(ones1[:], 1.0)
        iota = const.tile([WIN, WIN], F32)
        nc.gpsimd.iota(
            iota[:],
            pattern=[[1, WIN]],
            base=0,
            channel_multiplier=0,
            allow_small_or_imprecise_dtypes=True,
        )

        hT = const.tile([H, NW * WIN], BF16)

        # z shards / halo-exchange buffers, viewed as bf16 row-pairs
        zshA = dram.tile([p.rows_a // 2, 2 * O], BF16)
        zshB = dram.tile([p.rows_b // 2, 2 * O], BF16)
        zfullA = dram.tile([p.pairs_a, 2 * O], BF16, addr_space="Shared")
        zfullB = dram.tile([p.pairs_b, 2 * O], BF16, addr_space="Shared")

        def gather_slab(pool, ph, g, r, src_ap, elem, tag, eidx_s):
            s, n = ph.slab_meta[g][r]
            if n == 0:
                return None, s
            t = pool.tile([WIN, ph.max_slab[r], elem], BF16, tag=tag)
            for gi, i, nb, cs in ph.by_slab[(g, r)]:
                nc.gpsimd.dma_gather(
                    out_ap=t[:, i : i + nb, :],
                    in_ap=src_ap,
                    idxs_ap=eidx_s[:, cs * 8 : (cs + nb) * 8],
                    num_idxs=nb * WIN,
                    num_idxs_reg=nb * WIN,
                    elem_size=elem,
                    queue_num=next_q(),
                    single_packet=(nb <= 8),
                )
            return t, s

        def build_eq(pool, ph, ecode_s, g):
            """Batched 0/1 one-hot for all windows of group g: [WIN, n_g, WIN]."""
            g0, g1 = ph.groups[g]
            n = ph.group_nw[g]
            c0 = int(ph.wc0[g0])
            eq = pool.tile([WIN, ph.max_group_nw, WIN], BF16, tag="eq")
            nc.vector.tensor_tensor(
                out=eq[:, :n, :],
                in0=ecode_s[:, c0 : c0 + n].to_broadcast([WIN, n, WIN]),
                in1=iota[:].rearrange("p f -> p () f").to_broadcast([WIN, n, WIN]),
                op=AOT.is_equal,
            )
            return eq, c0

        # ---------------- phase 1 ----------------
        with (
            tc.tile_pool(name="gix1", bufs=1) as gixp,
            tc.tile_pool(name="xg", bufs=GATHER_BUFS) as xgp,
            tc.tile_pool(name="oh1", bufs=2) as ohp,
            tc.tile_pool(name="srg", bufs=2) as srp,
            tc.tile_pool(name="aggn", bufs=2) as aggp,
            tc.tile_pool(name="psA", bufs=2, space="PSUM") as psA,
            tc.tile_pool(name="psH", bufs=2, space="PSUM") as psH,
            tc.tile_pool(name="psZ", bufs=2, space="PSUM") as psZ,
        ):
            eidx1_s = gixp.tile([WIN, ph1.C * 8], I16)
            nc.sync.dma_start(eidx1_s[:], eidx1_d)

            def load_sr(g):
                g0, g1 = groups[g]
                t = srp.tile([WIN, GROUP * WIN], F32, tag="sr")
                nc.scalar.dma_start(
                    t[:, : (g1 - g0) * WIN], sr_d[:, g0 * WIN : g1 * WIN]
                )
                return t

            eq_tiles = {0: build_eq(ohp, ph1, ecode1_s, 0)}
            sr_tiles = {0: load_sr(0)}

            for g, (g0, g1) in enumerate(groups):
                slabs = {}
                for r in range(ph1.nranges):
                    r0 = r * RSZ1
                    r1 = min(r0 + RSZ1, p.n_nodes)
                    t, s = gather_slab(
                        xgp, ph1, g, r, x_d[r0:r1, :], F, f"xg{r}", eidx1_s
                    )
                    if t is not None:
                        slabs[r] = (t, s)

                if g + 1 < ngroups:
                    eq_tiles[g + 1] = build_eq(ohp, ph1, ecode1_s, g + 1)
                    sr_tiles[g + 1] = load_sr(g + 1)
                eq, eq_c0 = eq_tiles.pop(g)
                srg = sr_tiles.pop(g)

                for w in range(g0, g1):
                    rows = p.rows_last if w == NW - 1 else WIN
                    wsl = slice(w * WIN, (w + 1) * WIN)
                    chunks = ph1.window_chunks[w]
                    wcol = int(ph1.wc0[w]) - eq_c0

                    agg = psA.tile([F, WIN], F32, tag="agg")
                    for jj, (r, gid) in enumerate(chunks):
                        t, s = slabs[r]
                        nc.tensor.matmul(
                            out=agg[:],
                            lhsT=t[:, gid - s, :],
                            rhs=eq[:, wcol + jj, :],
                            start=(jj == 0),
                            stop=(jj == len(chunks) - 1),
                        )

                    # aggn = (agg * s_in[dst]) in bf16 (PSUM -> SBUF)
                    aggn = aggp.tile([F, WIN], BF16, tag="aggn")
                    nc.vector.tensor_tensor(
                        out=aggn[:],
                        in0=agg[:],
                        in1=srg[:, (w - g0) * WIN : (w - g0 + 1) * WIN],
                        op=AOT.mult,
                    )

                    hpre = psH.tile([H, WIN], F32, tag="hpre")
                    nc.tensor.matmul(
                        out=hpre[:], lhsT=W1s[:], rhs=aggn[:], start=True, stop=True
                    )
                    nc.scalar.activation(hT[:, wsl], hpre[:], AFT.Relu, bias=b1s[:])

                    zp = psZ.tile([WIN, O], F32, tag="zp")
                    nc.tensor.matmul(
                        out=zp[:], lhsT=hT[:, wsl], rhs=Wns[:], start=True, stop=True
                    )
                    zt = aggp.tile([WIN, O], BF16, tag="zt")
                    nc.scalar.activation(zt[:], zp[:], AFT.Copy)
                    if w < HALF_W:
                        nc.sync.dma_start(
                            zshA[w * (WIN // 2) : w * (WIN // 2) + rows // 2, :],
                            zt[:rows, :],
                        )
                    else:
                        wb = w - HALF_W
                        nc.sync.dma_start(
                            zshB[wb * (WIN // 2) : wb * (WIN // 2) + rows // 2, :],
                            zt[:rows, :],
                        )
        # ---------------- halo exchange (split; A can fire while phase-1
        # MMs drain, B after the full shard is written) ----
        nc.gpsimd.collective_compute(
            "AllGather", AOT.bypass,
            replica_groups=[list(range(p.n_cores))],
            ins=[zshA.opt()], outs=[zfullA.opt()],
        )
        nc.gpsimd.collective_compute(
            "AllGather", AOT.bypass,
            replica_groups=[list(range(p.n_cores))],
            ins=[zshB.opt()], outs=[zfullB.opt()],
        )

        # ---------------- phase 2 ----------------
        with (
            tc.tile_pool(name="gix2", bufs=1) as gixp2,
            tc.tile_pool(name="zg", bufs=GATHER_BUFS) as zgp,
            tc.tile_pool(name="oh2", bufs=2) as ohp2,
            tc.tile_pool(name="nm", bufs=2) as nmp,
            tc.tile_pool(name="psN", bufs=2, space="PSUM") as psN,
            tc.tile_pool(name="psS", bufs=2, space="PSUM") as psS,
        ):
            eidx2_s = gixp2.tile([WIN, ph2.C * 8], I16)
            nc.sync.dma_start(eidx2_s[:], eidx2_d)

            eq_tiles = {0: build_eq(ohp2, ph2, ecode2_s, 0)}
            zsrc = [zfullA.opt(), zfullB.opt()]
            # prefetch range-A gathers (they only need the first allgather
            # half) ahead of the first range-B gather, which blocks the
            # gpsimd queue until the second allgather completes
            pref = GATHER_BUFS
            rA = {}
            for g in range(min(pref, ngroups)):
                rA[g] = gather_slab(
                    zgp, ph2, g, 0, zsrc[0], 2 * O, "zg0", eidx2_s
                )
            for g, (g0, g1) in enumerate(groups):
                slabs = {}
                t, s = rA.pop(g)
                if t is not None:
                    slabs[0] = (t, s)
                t, s = gather_slab(
                    zgp, ph2, g, 1, zsrc[1], 2 * O, "zg1", eidx2_s
                )
                if t is not None:
                    slabs[1] = (t, s)
                if g + pref < ngroups:
                    rA[g + pref] = gather_slab(
                        zgp, ph2, g + pref, 0, zsrc[0], 2 * O, "zg0", eidx2_s
                    )

                if g + 1 < ngroups:
                    eq_tiles[g + 1] = build_eq(ohp2, ph2, ecode2_s, g + 1)
                eq, eq_c0 = eq_tiles.pop(g)

                for w in range(g0, g1):
                    rows = p.rows_last if w == NW - 1 else WIN
                    wsl = slice(w * WIN, (w + 1) * WIN)
                    chunks = ph2.window_chunks[w]
                    wcol = int(ph2.wc0[w]) - eq_c0

                    nm = psN.tile([WIN, O], F32, tag="nm")
                    for jj, (cl, gid) in enumerate(chunks):
                        r, par = cl >> 1, cl & 1
                        t, s = slabs[r]
                        nc.tensor.matmul(
                            out=nm[:],
                            lhsT=eq[:, wcol + jj, :],
                            rhs=t[:, gid - s, par * O : (par + 1) * O],
                            start=(jj == 0),
                            stop=(jj == len(chunks) - 1),
                        )

                    sb = psS.tile([WIN, O], F32, tag="sb")
                    nc.tensor.matmul(
                        out=sb[:], lhsT=ones1[:], rhs=b2s[:], start=True, stop=False
                    )
                    nc.tensor.matmul(
                        out=sb[:], lhsT=hT[:, wsl], rhs=Wss[:], start=False, stop=True
                    )

                    nms = nmp.tile([WIN, O], F32, tag="nms")
                    nc.vector.tensor_scalar(
                        out=nms[:], in0=nm[:], scalar1=invd_s[:, w : w + 1],
                        scalar2=None, op0=AOT.mult,
                    )
                    outt = nmp.tile([WIN, O], F32, tag="outt")
                    nc.vector.tensor_tensor(outt[:], nms[:], sb[:], op=AOT.add)
                    nc.sync.dma_start(
                        out_d[w * WIN : w * WIN + rows, :], outt[:rows, :]
                    )

    nc.compile()
    return nc


# ---------------------------------------------------------------------------
# Entry point
# ---------------------------------------------------------------------------

_CACHE = {}


def _get_compiled(p, F, H, O):
    key = (p.n_nodes, p.n_cores, p.ph1.C, p.ph2.C, F, H, O)
    if key not in _CACHE:
        import time as _time

        t0 = _time.time()
        _CACHE[key] = build_gcn(p, F, H, O)
        if os.environ.get("GCN_VERBOSE"):
            print(f"[gcn] build+bass-compile: {_time.time() - t0:.1f}s", flush=True)
    return _CACHE[key]


def make_in_maps(p, inputs):
    H = np.asarray(inputs["W1"]).shape[1]
    O = np.asarray(inputs["W_self"]).shape[1]
    x = np.asarray(inputs["x"], np.float32)
    xb = (x * p.s_out[:, None]).astype(BF)
    base = dict(
        x=np.ascontiguousarray(xb),
        W1=np.ascontiguousarray(np.asarray(inputs["W1"], np.float32).astype(BF)),
        b1=np.ascontiguousarray(np.asarray(inputs["b1"], np.float32).reshape(H, 1)),
        W_self=np.ascontiguousarray(np.asarray(inputs["W_self"], np.float32).astype(BF)),
        W_neigh=np.ascontiguousarray(np.asarray(inputs["W_neigh"], np.float32).astype(BF)),
        b2=np.ascontiguousarray(np.asarray(inputs["b2"], np.float32).reshape(1, O).astype(BF)),
    )
    in_maps = []
    for k in range(p.n_cores):
        m = dict(base)
        m.update(p.per_core[k])
        in_maps.append(m)
    return in_maps


def kernel(**inputs):
    x = np.asarray(inputs["x"])
    src = np.asarray(inputs["src"])
    dst = np.asarray(inputs["dst"])
    n_nodes, F = x.shape
    H = np.asarray(inputs["W1"]).shape[1]
    O = np.asarray(inputs["W_self"]).shape[1]

    p = prepare(src, dst, n_nodes)
    nc = _get_compiled(p, F, H, O)
    in_maps = make_in_maps(p, inputs)
    res = run_bass_kernel_spmd(
        nc, in_maps, core_ids=list(range(p.n_cores)),
        trace=bool(int(os.environ.get("GCN_TRACE", "0"))),
    )
    if os.environ.get("GCN_RESULT_HOOK"):
        _CACHE["last_results"] = res
    out = np.concatenate([r["out"] for r in res.results], axis=0)
    return out.astype(np.float32)


# revision 13
# speedup vs baseline: 1.0494x; 1.0494x over previous
"""GCN (GraphConv norm='both' -> ReLU -> SAGEConv mean) on 8 Trainium2 NeuronCores.

Contract: kernel(**inputs) takes the FULL inputs from setup_inputs() and
returns the FULL [N, OUT] output.

Sharding strategy (graph/data parallel, per the problem's sharding hint):
  - Nodes are partitioned contiguously across the 8 cores (12500 each).
  - Edges are partitioned by the owner of their *dst* node; each core's
    edges are bucketed per 128-node dst window into padded 128-edge chunks
    and aggregated with one-hot matmuls on the TensorEngine.
  - All gathered/streamed feature data is bf16 (tolerance is 2e-2; the
    bf16 pipeline sims at ~4e-3), halving the dominant cost: the random
    256-byte-per-edge dma_gather traffic, and running the one-hot matmuls
    at full PE rate (fp32 matmul is 4 cyc/row, bf16 is 1).
  - The degree normalization is split so the one-hot matrices stay pure
    0/1 (one DVE is_equal per group instead of two ops): s_out[src] is
    folded into x on the host (xb = s_out*x in bf16), and s_in[dst] is a
    per-dst-column scale fused into the PSUM->SBUF copy before W1.
  - Phase 1 (per core): dma_gather xb rows (4 int16-index ranges),
    one-hot matmul segment-sum into PSUM, hT = relu(W1.T@(agg*s_in)+b1)
    kept SBUF-resident in bf16, z = h @ W_neigh written (bf16) to a local
    z shard viewed as row-pairs.
  - Halo exchange: z (bf16, 64 wide = 12.8 MB total) is AllGathered in
    TWO halves so the first collective overlaps the second half of
    phase 1.  Nodes are remapped (host-side) so each half is rank-major
    contiguous.
  - Phase 2 (per core): dma_gather z-row-PAIRS (256B descriptors -- the
    gather elem must be a multiple of 256B, so single 128B bf16 z rows
    cannot be gathered directly).  Host packs edges into parity-pure
    chunks so each chunk's matmul reads the correct 64-column half of the
    gathered pair.  Segment-sum with 0/1 one-hots, scale by 1/deg_in,
    add h @ W_self + b2, write the core's [12500, 64] fp32 output shard.
  - Host concatenates the 8 shards.

Engine assignment is chosen to avoid FIFO head-of-line serialization:
the full int16 gather-index arrays are preloaded into SBUF (no per-group
index DMAs on the Sync engine), all PSUM->SBUF copies run on the Scalar
engine, and the Vector engine runs only the one-hot builds (software-
pipelined one group ahead) plus small per-window scales.
"""

import os
import sys
from contextlib import ExitStack

import numpy as np
import ml_dtypes

BF = ml_dtypes.bfloat16

for _p in ("/opt/trn_rl_repo", "/opt/pypackages"):
    if _p not in sys.path:
        sys.path.append(_p)

import concourse.bacc as bacc
import concourse.bass as bass
import concourse.mybir as mybir
import concourse.tile as tile
from concourse.bass_utils import run_bass_kernel_spmd

F32 = mybir.dt.float32
BF16 = mybir.dt.bfloat16
I16 = mybir.dt.int16
AOT = mybir.AluOpType
AFT = mybir.ActivationFunctionType

N_CORES = 8
WIN = 128
RSZ1 = 32768          # phase-1 src index range (int16 limit)
GROUP = 4             # windows per gather/eq group
SUBCHUNKS = int(os.environ.get("GCN_SUB", "64"))   # max chunks per dma_gather
NQUEUES = 4
HALF_W = 49           # windows per allgather half (49*128 = 6272 rows)
GATHER_BUFS = int(os.environ.get("GCN_GB", "4"))
SCRATCH = int(os.environ.get("GCN_SCRATCH", "16384"))  # SWDGE desc carveout B/partition


def _install_ntff_hook_shim():
    """The agent image's antenv lacks axon_hooks; provide it so trace=True
    can capture NTFF profiles through libaxon."""
    try:
        from antenv import axon_hooks  # noqa: F401
        return
    except ImportError:
        pass
    try:
        import types

        import antenv
        from trn_agent_boot.trn_boot import _ntff_profile_via_ctypes

        mod = types.ModuleType("antenv.axon_hooks")
        mod._hook = _ntff_profile_via_ctypes("/opt/axon/libaxon_pjrt.so")

        def get_axon_ntff_profile_hook():
            return mod._hook

        def set_axon_ntff_profile_hook(h):
            mod._hook = h

        mod.get_axon_ntff_profile_hook = get_axon_ntff_profile_hook
        mod.set_axon_ntff_profile_hook = set_axon_ntff_profile_hook
        sys.modules["antenv.axon_hooks"] = mod
        antenv.axon_hooks = mod
    except Exception:
        pass


_install_ntff_hook_shim()


# ---------------------------------------------------------------------------
# Host-side graph prep
# ---------------------------------------------------------------------------

class Phase:
    """Chunked edge-bucket structure for one gather/segment-sum phase."""
    pass


def _build_phase(owner, wrow, code, idx_local, cell_of, ncells, range_of_cell,
                 n_cores, NW, sub):
    """Bucket edges into per-(window, cell) 128-slot chunks, padded to the
    max count over cores so the SPMD program is identical on all cores.

    cell_of: per-edge cell id in [0, ncells); range_of_cell: gather source
    range per cell (cells sharing a range share a gather slab).
    """
    nranges = int(max(range_of_cell)) + 1
    counts = np.zeros((n_cores, NW, ncells), np.int64)
    np.add.at(counts, (owner, wrow, cell_of), 1)
    cwr = (counts.max(axis=0) + WIN - 1) // WIN          # [NW, ncells]
    empty = cwr.sum(axis=1) == 0
    cwr[empty, 0] = 1
    n_w = cwr.sum(axis=1)

    groups = [(g0, min(g0 + GROUP, NW)) for g0 in range(0, NW, GROUP)]

    # gather order: group -> range -> window -> cell(in range) -> chunk
    cell_start = np.zeros((NW, ncells), np.int64)
    slab_meta = []                                       # [g][r] = (start, n)
    c = 0
    for g0, g1 in groups:
        metas = []
        for r in range(nranges):
            s = c
            for w in range(g0, g1):
                for cl in range(ncells):
                    if range_of_cell[cl] != r:
                        continue
                    cell_start[w, cl] = c
                    c += int(cwr[w, cl])
            metas.append((s, c - s))
        slab_meta.append(metas)
    C = c

    gathers = []                # (g, r, chunk_off_in_slab, nb, global_chunk)
    if os.environ.get("GCN_CELLGATHER", "0") == "1":
        # one gather per (window, cell): pad slots carry trailing -1 indices,
        # which the SWDGE gather ucode skips (no descriptors generated)
        for g, (g0, g1) in enumerate(groups):
            for r in range(nranges):
                s, n = slab_meta[g][r]
                for w in range(g0, g1):
                    for cl in range(ncells):
                        if range_of_cell[cl] != r:
                            continue
                        cs = int(cell_start[w, cl])
                        nb = int(cwr[w, cl])
                        for i in range(0, nb, sub):
                            nbb = min(sub, nb - i)
                            gathers.append((g, r, cs - s + i, nbb, cs + i))
    else:
        for g in range(len(groups)):
            for r in range(nranges):
                s, n = slab_meta[g][r]
                for i in range(0, n, sub):
                    nb = min(sub, n - i)
                    gathers.append((g, r, i, nb, s + i))

    # window-major chunk columns (for the one-hot code arrays)
    wc0 = np.zeros(NW, np.int64)
    wc0[1:] = np.cumsum(n_w)[:-1]

    window_chunks = []          # [w] -> list of (cell, gather_chunk_id)
    for w in range(NW):
        lst = []
        for cl in range(ncells):
            for j in range(int(cwr[w, cl])):
                lst.append((cl, int(cell_start[w, cl]) + j))
        window_chunks.append(lst)

    per_core = []
    for k in range(n_cores):
        m = owner == k
        key = (wrow[m] * ncells + cell_of[m]).astype(np.int64)
        order = np.argsort(key, kind="stable")
        key = key[order]
        e_idx = idx_local[m][order]
        e_code = code[m][order]
        bounds = np.searchsorted(key, np.arange(NW * ncells + 1))

        cellgather = os.environ.get("GCN_CELLGATHER", "0") == "1"
        A_idx = np.zeros(C * WIN, np.int16)
        W_code = np.full(C * WIN, 255.0, np.float32)
        for w in range(NW):
            woff = 0
            for cl in range(ncells):
                a, b = bounds[w * ncells + cl], bounds[w * ncells + cl + 1]
                n = b - a
                gbase = int(cell_start[w, cl]) * WIN
                wbase = (int(wc0[w]) + woff) * WIN
                woff += int(cwr[w, cl])
                if n > 0:
                    A_idx[gbase : gbase + n] = e_idx[a:b].astype(np.int16)
                    W_code[wbase : wbase + n] = e_code[a:b]
                if cellgather:
                    # trailing pads generate no DMA descriptors; keep >=128
                    # valid slots (a full partition sweep) so every SDMA
                    # engine gets a descriptor and increments the sem
                    slots = int(cwr[w, cl]) * WIN
                    A_idx[gbase + max(n, 128) : gbase + slots] = -1

        eidx = np.ascontiguousarray(np.tile(A_idx.reshape(C * 8, 16).T, (8, 1)))
        ecode = np.ascontiguousarray(W_code.reshape(C, WIN).T)
        per_core.append((eidx, ecode))

    ph = Phase()
    ph.nranges = nranges
    ph.cwr = cwr
    ph.n_w = n_w
    ph.wc0 = wc0
    ph.C = C
    ph.groups = groups
    ph.slab_meta = slab_meta
    ph.gathers = gathers
    ph.window_chunks = window_chunks
    ph.per_core = per_core
    ph.max_nw = int(n_w.max())
    ph.group_nw = [int(n_w[g0:g1].sum()) for g0, g1 in groups]
    ph.max_group_nw = max(ph.group_nw)
    ph.group_c0 = [slab_meta[g][0][0] for g in range(len(groups))]
    ph.group_c1 = ph.group_c0[1:] + [C]
    ph.max_slab = [
        max((slab_meta[g][r][1] for g in range(len(groups))), default=0)
        for r in range(nranges)
    ]
    by_slab = {}
    for gi, (g, r, i, nb, cs) in enumerate(gathers):
        by_slab.setdefault((g, r), []).append((gi, i, nb, cs))
    ph.by_slab = by_slab
    return ph


class Prep:
    pass


def prepare(src, dst, n_nodes, n_cores=N_CORES):
    src = np.asarray(src).astype(np.int64)
    dst = np.asarray(dst).astype(np.int64)
    P = n_nodes // n_cores
    assert P * n_cores == n_nodes
    NW = (P + WIN - 1) // WIN
    rows_last = P - WIN * (NW - 1)
    rows_a = HALF_W * WIN                # 6272 local rows in allgather half A
    rows_b = P - rows_a
    assert rows_a % 2 == 0 and rows_b % 2 == 0

    deg_out = np.bincount(src, minlength=n_nodes).astype(np.float32)
    deg_in = np.bincount(dst, minlength=n_nodes).astype(np.float32)
    s_out = np.where(deg_out > 0, 1.0 / np.sqrt(np.maximum(deg_out, 1.0)), 0.0)
    s_in = np.where(deg_in > 0, 1.0 / np.sqrt(np.maximum(deg_in, 1.0)), 0.0)
    invd = (1.0 / np.maximum(deg_in, 1.0)).astype(np.float32)

    owner = dst // P
    ldst = dst - owner * P
    wrow = ldst // WIN
    code = (ldst % WIN).astype(np.float32)

    # ---- phase 1: gather xb rows; cells = 4 int16 src ranges ----
    rng1 = np.minimum(src // RSZ1, 3)
    idx1 = src - rng1 * RSZ1
    ph1 = _build_phase(owner, wrow, code, idx1, rng1, 4, [0, 1, 2, 3],
                       n_cores, NW, SUBCHUNKS)

    # ---- phase 2: gather z row-pairs from the remapped (split-allgather)
    # z layout; cells = (pair-range A/B) x (row parity) ----
    sc = src // P
    sl = src - sc * P
    new_row = np.where(sl < rows_a,
                       sc * rows_a + sl,
                       n_cores * rows_a + sc * rows_b + (sl - rows_a))
    pairs_a = n_cores * rows_a // 2      # 25088
    pr = new_row >> 1
    parity = (new_row & 1).astype(np.int64)
    r2 = (pr >= pairs_a).astype(np.int64)
    idx2 = pr - r2 * pairs_a
    cell2 = r2 * 2 + parity
    ph2 = _build_phase(owner, wrow, code, idx2, cell2, 4, [0, 0, 1, 1],
                       n_cores, NW, SUBCHUNKS)

    per_core = []
    for k in range(n_cores):
        eidx1, ecode1 = ph1.per_core[k]
        eidx2, ecode2 = ph2.per_core[k]
        nodes = np.arange(P) + k * P
        iv = np.zeros(NW * WIN, np.float32)
        iv[:P] = invd[nodes]
        sr = np.zeros(NW * WIN, np.float32)
        sr[:P] = s_in[nodes]
        per_core.append(dict(
            eidx1=eidx1, ecode1=ecode1,
            eidx2=eidx2, ecode2=ecode2,
            invd=np.ascontiguousarray(iv.reshape(NW, WIN).T),
            sr=np.ascontiguousarray(np.broadcast_to(sr, (WIN, NW * WIN))),
        ))

    p = Prep()
    p.P, p.NW, p.rows_last = P, NW, rows_last
    p.rows_a, p.rows_b = rows_a, rows_b
    p.pairs_a = pairs_a
    p.pairs_b = (n_nodes - n_cores * rows_a) // 2
    p.ph1, p.ph2 = ph1, ph2
    p.per_core = per_core
    p.s_out = s_out
    p.n_nodes = n_nodes
    p.n_cores = n_cores
    return p


# ---------------------------------------------------------------------------
# Bass/Tile kernel builder
# ---------------------------------------------------------------------------

def build_gcn(p, F, H, O):
    NW, P = p.NW, p.P
    ph1, ph2 = p.ph1, p.ph2
    groups = ph1.groups
    ngroups = len(groups)

    nc = bacc.Bacc(
        "TRN2", debug=False, enable_asserts=False, num_devices=p.n_cores,
        num_swdge_queues=NQUEUES, dynamic_dma_scratch_size=SCRATCH,
    )

    x_d = nc.dram_tensor("x", [p.n_nodes, F], BF16, kind="ExternalInput").ap()
    W1_d = nc.dram_tensor("W1", [F, H], BF16, kind="ExternalInput").ap()
    b1_d = nc.dram_tensor("b1", [H, 1], F32, kind="ExternalInput").ap()
    Ws_d = nc.dram_tensor("W_self", [H, O], BF16, kind="ExternalInput").ap()
    Wn_d = nc.dram_tensor("W_neigh", [H, O], BF16, kind="ExternalInput").ap()
    b2_d = nc.dram_tensor("b2", [1, O], BF16, kind="ExternalInput").ap()
    eidx1_d = nc.dram_tensor("eidx1", [WIN, ph1.C * 8], I16, kind="ExternalInput").ap()
    ecode1_d = nc.dram_tensor("ecode1", [WIN, ph1.C], F32, kind="ExternalInput").ap()
    eidx2_d = nc.dram_tensor("eidx2", [WIN, ph2.C * 8], I16, kind="ExternalInput").ap()
    ecode2_d = nc.dram_tensor("ecode2", [WIN, ph2.C], F32, kind="ExternalInput").ap()
    invd_d = nc.dram_tensor("invd", [WIN, NW], F32, kind="ExternalInput").ap()
    sr_d = nc.dram_tensor("sr", [WIN, NW * WIN], F32, kind="ExternalInput").ap()
    out_d = nc.dram_tensor("out", [P, O], F32, kind="ExternalOutput").ap()

    qn = [0]

    def next_q():
        q = qn[0]
        qn[0] = (q + 1) % NQUEUES
        return q

    with tile.TileContext(nc, num_cores=p.n_cores) as tc, ExitStack() as ctx:
        const = ctx.enter_context(tc.tile_pool(name="const", bufs=1))
        dram = ctx.enter_context(tc.tile_pool(name="dram", bufs=1, space="DRAM"))

        W1s = const.tile([F, H], BF16)
        nc.sync.dma_start(W1s[:], W1_d)
        Wss = const.tile([H, O], BF16)
        nc.sync.dma_start(Wss[:], Ws_d)
        Wns = const.tile([H, O], BF16)
        nc.sync.dma_start(Wns[:], Wn_d)
        b1s = const.tile([H, 1], F32)
        nc.sync.dma_start(b1s[:], b1_d)
        b2s = const.tile([1, O], BF16)
        nc.sync.dma_start(b2s[:], b2_d)
        ecode1_s = const.tile([WIN, ph1.C], F32)
        nc.sync.dma_start(ecode1_s[:], ecode1_d)
        ecode2_s = const.tile([WIN, ph2.C], F32)
        nc.sync.dma_start(ecode2_s[:], ecode2_d)
        invd_s = const.tile([WIN, NW], F32)
        nc.sync.dma_start(invd_s[:], invd_d)

        ones1 = const.tile([1, WIN], BF16)
        nc.vector.memset(ones1[:], 1.0)
        iota = const.tile([WIN, WIN], F32)
        nc.gpsimd.iota(
            iota[:],
            pattern=[[1, WIN]],
            base=0,
            channel_multiplier=0,
            allow_small_or_imprecise_dtypes=True,
        )

        hT = const.tile([H, NW * WIN], BF16)

        # z shards / halo-exchange buffers, viewed as bf16 row-pairs
        zshA = dram.tile([p.rows_a // 2, 2 * O], BF16)
        zshB = dram.tile([p.rows_b // 2, 2 * O], BF16)
        zfullA = dram.tile([p.pairs_a, 2 * O], BF16, addr_space="Shared")
        zfullB = dram.tile([p.pairs_b, 2 * O], BF16, addr_space="Shared")

        def gather_slab(pool, ph, g, r, src_ap, elem, tag, eidx_s):
            s, n = ph.slab_meta[g][r]
            if n == 0:
                return None, s
            t = pool.tile([WIN, ph.max_slab[r], elem], BF16, tag=tag)
            for gi, i, nb, cs in ph.by_slab[(g, r)]:
                nc.gpsimd.dma_gather(
                    out_ap=t[:, i : i + nb, :],
                    in_ap=src_ap,
                    idxs_ap=eidx_s[:, cs * 8 : (cs + nb) * 8],
                    num_idxs=nb * WIN,
                    num_idxs_reg=nb * WIN,
                    elem_size=elem,
                    queue_num=next_q(),
                    single_packet=(nb <= 8),
                )
            return t, s

        def build_eq(pool, ph, ecode_s, g):
            """Batched 0/1 one-hot for all windows of group g: [WIN, n_g, WIN]."""
            g0, g1 = ph.groups[g]
            n = ph.group_nw[g]
            c0 = int(ph.wc0[g0])
            eq = pool.tile([WIN, ph.max_group_nw, WIN], BF16, tag="eq")
            nc.vector.tensor_tensor(
                out=eq[:, :n, :],
                in0=ecode_s[:, c0 : c0 + n].to_broadcast([WIN, n, WIN]),
                in1=iota[:].rearrange("p f -> p () f").to_broadcast([WIN, n, WIN]),
                op=AOT.is_equal,
            )
            return eq, c0

        # ---------------- phase 1 ----------------
        with (
            tc.tile_pool(name="gix1", bufs=1) as gixp,
            tc.tile_pool(name="xg", bufs=GATHER_BUFS) as xgp,
            tc.tile_pool(name="oh1", bufs=2) as ohp,
            tc.tile_pool(name="srg", bufs=2) as srp,
            tc.tile_pool(name="aggn", bufs=2) as aggp,
            tc.tile_pool(name="psA", bufs=2, space="PSUM") as psA,
            tc.tile_pool(name="psH", bufs=2, space="PSUM") as psH,
            tc.tile_pool(name="psZ", bufs=2, space="PSUM") as psZ,
        ):
            eidx1_s = gixp.tile([WIN, ph1.C * 8], I16)
            nc.sync.dma_start(eidx1_s[:], eidx1_d)

            def load_sr(g):
                g0, g1 = groups[g]
                t = srp.tile([WIN, GROUP * WIN], F32, tag="sr")
                nc.scalar.dma_start(
                    t[:, : (g1 - g0) * WIN], sr_d[:, g0 * WIN : g1 * WIN]
                )
                return t

            eq_tiles = {0: build_eq(ohp, ph1, ecode1_s, 0)}
            sr_tiles = {0: load_sr(0)}

            for g, (g0, g1) in enumerate(groups):
                slabs = {}
                for r in range(ph1.nranges):
                    r0 = r * RSZ1
                    r1 = min(r0 + RSZ1, p.n_nodes)
                    t, s = gather_slab(
                        xgp, ph1, g, r, x_d[r0:r1, :], F, f"xg{r}", eidx1_s
                    )
                    if t is not None:
                        slabs[r] = (t, s)

                if g + 1 < ngroups:
                    eq_tiles[g + 1] = build_eq(ohp, ph1, ecode1_s, g + 1)
                    sr_tiles[g + 1] = load_sr(g + 1)
                eq, eq_c0 = eq_tiles.pop(g)
                srg = sr_tiles.pop(g)

                for w in range(g0, g1):
                    rows = p.rows_last if w == NW - 1 else WIN
                    wsl = slice(w * WIN, (w + 1) * WIN)
                    chunks = ph1.window_chunks[w]
                    wcol = int(ph1.wc0[w]) - eq_c0

                    agg = psA.tile([F, WIN], F32, tag="agg")
                    for jj, (r, gid) in enumerate(chunks):
                        t, s = slabs[r]
                        nc.tensor.matmul(
                            out=agg[:],
                            lhsT=t[:, gid - s, :],
                            rhs=eq[:, wcol + jj, :],
                            start=(jj == 0),
                            stop=(jj == len(chunks) - 1),
                        )

                    # aggn = (agg * s_in[dst]) in bf16 (PSUM -> SBUF)
                    aggn = aggp.tile([F, WIN], BF16, tag="aggn")
                    nc.vector.tensor_tensor(
                        out=aggn[:],
                        in0=agg[:],
                        in1=srg[:, (w - g0) * WIN : (w - g0 + 1) * WIN],
                        op=AOT.mult,
                    )

                    hpre = psH.tile([H, WIN], F32, tag="hpre")
                    nc.tensor.matmul(
                        out=hpre[:], lhsT=W1s[:], rhs=aggn[:], start=True, stop=True
                    )
                    nc.scalar.activation(hT[:, wsl], hpre[:], AFT.Relu, bias=b1s[:])

                    zp = psZ.tile([WIN, O], F32, tag="zp")
                    nc.tensor.matmul(
                        out=zp[:], lhsT=hT[:, wsl], rhs=Wns[:], start=True, stop=True
                    )
                    zt = aggp.tile([WIN, O], BF16, tag="zt")
                    nc.scalar.activation(zt[:], zp[:], AFT.Copy)
                    if w < HALF_W:
                        nc.sync.dma_start(
                            zshA[w * (WIN // 2) : w * (WIN // 2) + rows // 2, :],
                            zt[:rows, :],
                        )
                    else:
                        wb = w - HALF_W
                        nc.sync.dma_start(
                            zshB[wb * (WIN // 2) : wb * (WIN // 2) + rows // 2, :],
                            zt[:rows, :],
                        )
        # ---------------- halo exchange (split; A can fire while phase-1
        # MMs drain, B after the full shard is written) ----
        nc.gpsimd.collective_compute(
            "AllGather", AOT.bypass,
            replica_groups=[list(range(p.n_cores))],
            ins=[zshA.opt()], outs=[zfullA.opt()],
        )
        nc.gpsimd.collective_compute(
            "AllGather", AOT.bypass,
            replica_groups=[list(range(p.n_cores))],
            ins=[zshB.opt()], outs=[zfullB.opt()],
        )

        # ---------------- phase 2 ----------------
        with (
            tc.tile_pool(name="gix2", bufs=1) as gixp2,
            tc.tile_pool(name="zg", bufs=GATHER_BUFS) as zgp,
            tc.tile_pool(name="oh2", bufs=2) as ohp2,
            tc.tile_pool(name="nm", bufs=2) as nmp,
            tc.tile_pool(name="psN", bufs=2, space="PSUM") as psN,
            tc.tile_pool(name="psS", bufs=2, space="PSUM") as psS,
        ):
            eidx2_s = gixp2.tile([WIN, ph2.C * 8], I16)
            nc.sync.dma_start(eidx2_s[:], eidx2_d)

            eq_tiles = {0: build_eq(ohp2, ph2, ecode2_s, 0)}
            zsrc = [zfullA.opt(), zfullB.opt()]
            for g, (g0, g1) in enumerate(groups):
                slabs = {}
                for r in range(ph2.nranges):
                    t, s = gather_slab(
                        zgp, ph2, g, r, zsrc[r], 2 * O, f"zg{r}", eidx2_s
                    )
                    if t is not None:
                        slabs[r] = (t, s)

                if g + 1 < ngroups:
                    eq_tiles[g + 1] = build_eq(ohp2, ph2, ecode2_s, g + 1)
                eq, eq_c0 = eq_tiles.pop(g)

                for w in range(g0, g1):
                    rows = p.rows_last if w == NW - 1 else WIN
                    wsl = slice(w * WIN, (w + 1) * WIN)
                    chunks = ph2.window_chunks[w]
                    wcol = int(ph2.wc0[w]) - eq_c0

                    nm = psN.tile([WIN, O], F32, tag="nm")
                    for jj, (cl, gid) in enumerate(chunks):
                        r, par = cl >> 1, cl & 1
                        t, s = slabs[r]
                        nc.tensor.matmul(
                            out=nm[:],
                            lhsT=eq[:, wcol + jj, :],
                            rhs=t[:, gid - s, par * O : (par + 1) * O],
                            start=(jj == 0),
                            stop=(jj == len(chunks) - 1),
                        )

                    sb = psS.tile([WIN, O], F32, tag="sb")
                    nc.tensor.matmul(
                        out=sb[:], lhsT=ones1[:], rhs=b2s[:], start=True, stop=False
                    )
                    nc.tensor.matmul(
                        out=sb[:], lhsT=hT[:, wsl], rhs=Wss[:], start=False, stop=True
                    )

                    nms = nmp.tile([WIN, O], F32, tag="nms")
                    nc.vector.tensor_scalar(
                        out=nms[:], in0=nm[:], scalar1=invd_s[:, w : w + 1],
                        scalar2=None, op0=AOT.mult,
                    )
                    outt = nmp.tile([WIN, O], F32, tag="outt")
                    nc.vector.tensor_tensor(outt[:], nms[:], sb[:], op=AOT.add)
                    nc.sync.dma_start(
                        out_d[w * WIN : w * WIN + rows, :], outt[:rows, :]
                    )

    nc.compile()
    return nc


# ---------------------------------------------------------------------------
# Entry point
# ---------------------------------------------------------------------------

_CACHE = {}


def _get_compiled(p, F, H, O):
    key = (p.n_nodes, p.n_cores, p.ph1.C, p.ph2.C, F, H, O)
    if key not in _CACHE:
        import time as _time

        t0 = _time.time()
        _CACHE[key] = build_gcn(p, F, H, O)
        if os.environ.get("GCN_VERBOSE"):
            print(f"[gcn] build+bass-compile: {_time.time() - t0:.1f}s", flush=True)
    return _CACHE[key]


def make_in_maps(p, inputs):
    H = np.asarray(inputs["W1"]).shape[1]
    O = np.asarray(inputs["W_self"]).shape[1]
    x = np.asarray(inputs["x"], np.float32)
    xb = (x * p.s_out[:, None]).astype(BF)
    base = dict(
        x=np.ascontiguousarray(xb),
        W1=np.ascontiguousarray(np.asarray(inputs["W1"], np.float32).astype(BF)),
        b1=np.ascontiguousarray(np.asarray(inputs["b1"], np.float32).reshape(H, 1)),
        W_self=np.ascontiguousarray(np.asarray(inputs["W_self"], np.float32).astype(BF)),
        W_neigh=np.ascontiguousarray(np.asarray(inputs["W_neigh"], np.float32).astype(BF)),
        b2=np.ascontiguousarray(np.asarray(inputs["b2"], np.float32).reshape(1, O).astype(BF)),
    )
    in_maps = []
    for k in range(p.n_cores):
        m = dict(base)
        m.update(p.per_core[k])
        in_maps.append(m)
    return in_maps


def kernel(**inputs):
    x = np.asarray(inputs["x"])
    src = np.asarray(inputs["src"])
    dst = np.asarray(inputs["dst"])
    n_nodes, F = x.shape
    H = np.asarray(inputs["W1"]).shape[1]
    O = np.asarray(inputs["W_self"]).shape[1]

    p = prepare(src, dst, n_nodes)
    nc = _get_compiled(p, F, H, O)
    in_maps = make_in_maps(p, inputs)
    res = run_bass_kernel_spmd(
        nc, in_maps, core_ids=list(range(p.n_cores)),
        trace=bool(int(os.environ.get("GCN_TRACE", "0"))),
    )
    if os.environ.get("GCN_RESULT_HOOK"):
        _CACHE["last_results"] = res
    out = np.concatenate([r["out"] for r in res.results], axis=0)
    return out.astype(np.float32)


# revision 14
# speedup vs baseline: 1.3352x; 1.2723x over previous
"""GCN (GraphConv norm='both' -> ReLU -> SAGEConv mean) on 8 Trainium2 NeuronCores.

Contract: kernel(**inputs) takes the FULL inputs from setup_inputs() and
returns the FULL [N, OUT] output.

Sharding strategy (graph/data parallel, per the problem's sharding hint):
  - Nodes are partitioned contiguously across the 8 cores (12500 each).
  - Edges are partitioned by the owner of their *dst* node; each core's
    edges are bucketed per 128-node dst window into padded 128-edge chunks
    and aggregated with one-hot matmuls on the TensorEngine.
  - All gathered/streamed feature data is bf16 (tolerance is 2e-2; the
    bf16 pipeline sims at ~4e-3), halving the dominant cost: the random
    256-byte-per-edge dma_gather traffic, and running the one-hot matmuls
    at full PE rate (fp32 matmul is 4 cyc/row, bf16 is 1).
  - The degree normalization is split so the one-hot matrices stay pure
    0/1 (one DVE is_equal per group instead of two ops): s_out[src] is
    folded into x on the host (xb = s_out*x in bf16), and s_in[dst] is a
    per-dst-column scale fused into the PSUM->SBUF copy before W1.
  - Phase 1 (per core): dma_gather xb rows (4 int16-index ranges),
    one-hot matmul segment-sum into PSUM, hT = relu(W1.T@(agg*s_in)+b1)
    kept SBUF-resident in bf16, z = h @ W_neigh written (bf16) to a local
    z shard viewed as row-pairs.
  - Halo exchange: z (bf16, 64 wide = 12.8 MB total) is AllGathered in
    TWO halves so the first collective overlaps the second half of
    phase 1.  Nodes are remapped (host-side) so each half is rank-major
    contiguous.
  - Phase 2 (per core): dma_gather z-row-PAIRS (256B descriptors -- the
    gather elem must be a multiple of 256B, so single 128B bf16 z rows
    cannot be gathered directly).  Host packs edges into parity-pure
    chunks so each chunk's matmul reads the correct 64-column half of the
    gathered pair.  Segment-sum with 0/1 one-hots, scale by 1/deg_in,
    add h @ W_self + b2, write the core's [12500, 64] fp32 output shard.
  - Host concatenates the 8 shards.

Engine assignment is chosen to avoid FIFO head-of-line serialization:
the full int16 gather-index arrays are preloaded into SBUF (no per-group
index DMAs on the Sync engine), all PSUM->SBUF copies run on the Scalar
engine, and the Vector engine runs only the one-hot builds (software-
pipelined one group ahead) plus small per-window scales.
"""

import os
import sys
from contextlib import ExitStack

import numpy as np
import ml_dtypes

BF = ml_dtypes.bfloat16

for _p in ("/opt/trn_rl_repo", "/opt/pypackages"):
    if _p not in sys.path:
        sys.path.append(_p)

import concourse.bacc as bacc
import concourse.bass as bass
import concourse.mybir as mybir
import concourse.tile as tile
from concourse.bass_utils import run_bass_kernel_spmd

F32 = mybir.dt.float32
BF16 = mybir.dt.bfloat16
I16 = mybir.dt.int16
AOT = mybir.AluOpType
AFT = mybir.ActivationFunctionType

N_CORES = 8
WIN = 128
RSZ1 = 32768          # phase-1 src index range (int16 limit)
GROUP = 4             # windows per gather/eq group
SUBCHUNKS = int(os.environ.get("GCN_SUB", "14"))   # max chunks per dma_gather
NQUEUES = 4
HALF_W = 49           # windows per allgather half (49*128 = 6272 rows)
GATHER_BUFS = int(os.environ.get("GCN_GB", "4"))
SCRATCH = int(os.environ.get("GCN_SCRATCH", "16384"))  # SWDGE desc carveout B/partition


def _install_ntff_hook_shim():
    """The agent image's antenv lacks axon_hooks; provide it so trace=True
    can capture NTFF profiles through libaxon."""
    try:
        from antenv import axon_hooks  # noqa: F401
        return
    except ImportError:
        pass
    try:
        import types

        import antenv
        from trn_agent_boot.trn_boot import _ntff_profile_via_ctypes

        mod = types.ModuleType("antenv.axon_hooks")
        mod._hook = _ntff_profile_via_ctypes("/opt/axon/libaxon_pjrt.so")

        def get_axon_ntff_profile_hook():
            return mod._hook

        def set_axon_ntff_profile_hook(h):
            mod._hook = h

        mod.get_axon_ntff_profile_hook = get_axon_ntff_profile_hook
        mod.set_axon_ntff_profile_hook = set_axon_ntff_profile_hook
        sys.modules["antenv.axon_hooks"] = mod
        antenv.axon_hooks = mod
    except Exception:
        pass


_install_ntff_hook_shim()


# ---------------------------------------------------------------------------
# Host-side graph prep
# ---------------------------------------------------------------------------

class Phase:
    """Chunked edge-bucket structure for one gather/segment-sum phase."""
    pass


def _build_phase(owner, wrow, code, idx_local, cell_of, ncells, range_of_cell,
                 n_cores, NW, sub):
    """Bucket edges into per-(window, cell) 128-slot chunks, padded to the
    max count over cores so the SPMD program is identical on all cores.

    cell_of: per-edge cell id in [0, ncells); range_of_cell: gather source
    range per cell (cells sharing a range share a gather slab).
    """
    nranges = int(max(range_of_cell)) + 1
    counts = np.zeros((n_cores, NW, ncells), np.int64)
    np.add.at(counts, (owner, wrow, cell_of), 1)
    cwr = (counts.max(axis=0) + WIN - 1) // WIN          # [NW, ncells]
    empty = cwr.sum(axis=1) == 0
    cwr[empty, 0] = 1
    n_w = cwr.sum(axis=1)

    groups = [(g0, min(g0 + GROUP, NW)) for g0 in range(0, NW, GROUP)]

    # gather order: group -> range -> window -> cell(in range) -> chunk
    cell_start = np.zeros((NW, ncells), np.int64)
    slab_meta = []                                       # [g][r] = (start, n)
    c = 0
    for g0, g1 in groups:
        metas = []
        for r in range(nranges):
            s = c
            for w in range(g0, g1):
                for cl in range(ncells):
                    if range_of_cell[cl] != r:
                        continue
                    cell_start[w, cl] = c
                    c += int(cwr[w, cl])
            metas.append((s, c - s))
        slab_meta.append(metas)
    C = c

    gathers = []                # (g, r, chunk_off_in_slab, nb, global_chunk)
    if os.environ.get("GCN_CELLGATHER", "0") == "1":
        # one gather per (window, cell): pad slots carry trailing -1 indices,
        # which the SWDGE gather ucode skips (no descriptors generated)
        for g, (g0, g1) in enumerate(groups):
            for r in range(nranges):
                s, n = slab_meta[g][r]
                for w in range(g0, g1):
                    for cl in range(ncells):
                        if range_of_cell[cl] != r:
                            continue
                        cs = int(cell_start[w, cl])
                        nb = int(cwr[w, cl])
                        for i in range(0, nb, sub):
                            nbb = min(sub, nb - i)
                            gathers.append((g, r, cs - s + i, nbb, cs + i))
    else:
        for g in range(len(groups)):
            for r in range(nranges):
                s, n = slab_meta[g][r]
                for i in range(0, n, sub):
                    nb = min(sub, n - i)
                    gathers.append((g, r, i, nb, s + i))

    # window-major chunk columns (for the one-hot code arrays)
    wc0 = np.zeros(NW, np.int64)
    wc0[1:] = np.cumsum(n_w)[:-1]

    window_chunks = []          # [w] -> list of (cell, gather_chunk_id)
    for w in range(NW):
        lst = []
        for cl in range(ncells):
            for j in range(int(cwr[w, cl])):
                lst.append((cl, int(cell_start[w, cl]) + j))
        window_chunks.append(lst)

    per_core = []
    for k in range(n_cores):
        m = owner == k
        key = (wrow[m] * ncells + cell_of[m]).astype(np.int64)
        order = np.argsort(key, kind="stable")
        key = key[order]
        e_idx = idx_local[m][order]
        e_code = code[m][order]
        bounds = np.searchsorted(key, np.arange(NW * ncells + 1))

        cellgather = os.environ.get("GCN_CELLGATHER", "0") == "1"
        A_idx = np.zeros(C * WIN, np.int16)
        W_code = np.full(C * WIN, 255.0, np.float32)
        for w in range(NW):
            woff = 0
            for cl in range(ncells):
                a, b = bounds[w * ncells + cl], bounds[w * ncells + cl + 1]
                n = b - a
                gbase = int(cell_start[w, cl]) * WIN
                wbase = (int(wc0[w]) + woff) * WIN
                woff += int(cwr[w, cl])
                if n > 0:
                    A_idx[gbase : gbase + n] = e_idx[a:b].astype(np.int16)
                    W_code[wbase : wbase + n] = e_code[a:b]
                if cellgather:
                    # trailing pads generate no DMA descriptors; keep >=128
                    # valid slots (a full partition sweep) so every SDMA
                    # engine gets a descriptor and increments the sem
                    slots = int(cwr[w, cl]) * WIN
                    A_idx[gbase + max(n, 128) : gbase + slots] = -1

        eidx = np.ascontiguousarray(np.tile(A_idx.reshape(C * 8, 16).T, (8, 1)))
        ecode = np.ascontiguousarray(W_code.reshape(C, WIN).T)
        per_core.append((eidx, ecode))

    ph = Phase()
    ph.nranges = nranges
    ph.cwr = cwr
    ph.n_w = n_w
    ph.wc0 = wc0
    ph.C = C
    ph.groups = groups
    ph.slab_meta = slab_meta
    ph.gathers = gathers
    ph.window_chunks = window_chunks
    ph.per_core = per_core
    ph.max_nw = int(n_w.max())
    ph.group_nw = [int(n_w[g0:g1].sum()) for g0, g1 in groups]
    ph.max_group_nw = max(ph.group_nw)
    ph.group_c0 = [slab_meta[g][0][0] for g in range(len(groups))]
    ph.group_c1 = ph.group_c0[1:] + [C]
    ph.max_slab = [
        max((slab_meta[g][r][1] for g in range(len(groups))), default=0)
        for r in range(nranges)
    ]
    by_slab = {}
    for gi, (g, r, i, nb, cs) in enumerate(gathers):
        by_slab.setdefault((g, r), []).append((gi, i, nb, cs))
    ph.by_slab = by_slab
    return ph


class Prep:
    pass


def prepare(src, dst, n_nodes, n_cores=N_CORES):
    src = np.asarray(src).astype(np.int64)
    dst = np.asarray(dst).astype(np.int64)
    P = n_nodes // n_cores
    assert P * n_cores == n_nodes
    NW = (P + WIN - 1) // WIN
    rows_last = P - WIN * (NW - 1)
    rows_a = HALF_W * WIN                # 6272 local rows in allgather half A
    rows_b = P - rows_a
    assert rows_a % 2 == 0 and rows_b % 2 == 0

    deg_out = np.bincount(src, minlength=n_nodes).astype(np.float32)
    deg_in = np.bincount(dst, minlength=n_nodes).astype(np.float32)
    s_out = np.where(deg_out > 0, 1.0 / np.sqrt(np.maximum(deg_out, 1.0)), 0.0)
    s_in = np.where(deg_in > 0, 1.0 / np.sqrt(np.maximum(deg_in, 1.0)), 0.0)
    invd = (1.0 / np.maximum(deg_in, 1.0)).astype(np.float32)

    owner = dst // P
    ldst = dst - owner * P
    wrow = ldst // WIN
    code = (ldst % WIN).astype(np.float32)

    # ---- phase 1: gather xb rows; cells = 4 int16 src ranges ----
    rng1 = np.minimum(src // RSZ1, 3)
    idx1 = src - rng1 * RSZ1
    ph1 = _build_phase(owner, wrow, code, idx1, rng1, 4, [0, 1, 2, 3],
                       n_cores, NW, SUBCHUNKS)

    # ---- phase 2: gather z row-pairs from the remapped (split-allgather)
    # z layout; cells = (pair-range A/B) x (row parity) ----
    sc = src // P
    sl = src - sc * P
    new_row = np.where(sl < rows_a,
                       sc * rows_a + sl,
                       n_cores * rows_a + sc * rows_b + (sl - rows_a))
    pairs_a = n_cores * rows_a // 2      # 25088
    pr = new_row >> 1
    parity = (new_row & 1).astype(np.int64)
    r2 = (pr >= pairs_a).astype(np.int64)
    idx2 = pr - r2 * pairs_a
    cell2 = r2 * 2 + parity
    ph2 = _build_phase(owner, wrow, code, idx2, cell2, 4, [0, 0, 1, 1],
                       n_cores, NW, SUBCHUNKS)

    per_core = []
    for k in range(n_cores):
        eidx1, ecode1 = ph1.per_core[k]
        eidx2, ecode2 = ph2.per_core[k]
        nodes = np.arange(P) + k * P
        iv = np.zeros(NW * WIN, np.float32)
        iv[:P] = invd[nodes]
        sr = np.zeros(NW * WIN, np.float32)
        sr[:P] = s_in[nodes]
        per_core.append(dict(
            eidx1=eidx1, ecode1=ecode1,
            eidx2=eidx2, ecode2=ecode2,
            invd=np.ascontiguousarray(iv.reshape(NW, WIN).T),
            sr=np.ascontiguousarray(np.broadcast_to(sr, (WIN, NW * WIN))),
        ))

    p = Prep()
    p.P, p.NW, p.rows_last = P, NW, rows_last
    p.rows_a, p.rows_b = rows_a, rows_b
    p.pairs_a = pairs_a
    p.pairs_b = (n_nodes - n_cores * rows_a) // 2
    p.ph1, p.ph2 = ph1, ph2
    p.per_core = per_core
    p.s_out = s_out
    p.n_nodes = n_nodes
    p.n_cores = n_cores
    return p


# ---------------------------------------------------------------------------
# Bass/Tile kernel builder
# ---------------------------------------------------------------------------

def build_gcn(p, F, H, O):
    NW, P = p.NW, p.P
    ph1, ph2 = p.ph1, p.ph2
    groups = ph1.groups
    ngroups = len(groups)

    nc = bacc.Bacc(
        "TRN2", debug=False, enable_asserts=False, num_devices=p.n_cores,
        num_swdge_queues=NQUEUES, dynamic_dma_scratch_size=SCRATCH,
    )

    x_d = nc.dram_tensor("x", [p.n_nodes, F], BF16, kind="ExternalInput").ap()
    W1_d = nc.dram_tensor("W1", [F, H], BF16, kind="ExternalInput").ap()
    b1_d = nc.dram_tensor("b1", [H, 1], F32, kind="ExternalInput").ap()
    Ws_d = nc.dram_tensor("W_self", [H, O], BF16, kind="ExternalInput").ap()
    Wn_d = nc.dram_tensor("W_neigh", [H, O], BF16, kind="ExternalInput").ap()
    b2_d = nc.dram_tensor("b2", [1, O], BF16, kind="ExternalInput").ap()
    eidx1_d = nc.dram_tensor("eidx1", [WIN, ph1.C * 8], I16, kind="ExternalInput").ap()
    ecode1_d = nc.dram_tensor("ecode1", [WIN, ph1.C], F32, kind="ExternalInput").ap()
    eidx2_d = nc.dram_tensor("eidx2", [WIN, ph2.C * 8], I16, kind="ExternalInput").ap()
    ecode2_d = nc.dram_tensor("ecode2", [WIN, ph2.C], F32, kind="ExternalInput").ap()
    invd_d = nc.dram_tensor("invd", [WIN, NW], F32, kind="ExternalInput").ap()
    sr_d = nc.dram_tensor("sr", [WIN, NW * WIN], F32, kind="ExternalInput").ap()
    out_d = nc.dram_tensor("out", [P, O], F32, kind="ExternalOutput").ap()

    qn = [0]

    def next_q():
        q = qn[0]
        qn[0] = (q + 1) % NQUEUES
        return q

    with tile.TileContext(nc, num_cores=p.n_cores) as tc, ExitStack() as ctx:
        const = ctx.enter_context(tc.tile_pool(name="const", bufs=1))
        dram = ctx.enter_context(tc.tile_pool(name="dram", bufs=1, space="DRAM"))

        W1s = const.tile([F, H], BF16)
        nc.sync.dma_start(W1s[:], W1_d)
        Wss = const.tile([H, O], BF16)
        nc.sync.dma_start(Wss[:], Ws_d)
        Wns = const.tile([H, O], BF16)
        nc.sync.dma_start(Wns[:], Wn_d)
        b1s = const.tile([H, 1], F32)
        nc.sync.dma_start(b1s[:], b1_d)
        b2s = const.tile([1, O], BF16)
        nc.sync.dma_start(b2s[:], b2_d)
        ecode1_s = const.tile([WIN, ph1.C], F32)
        nc.sync.dma_start(ecode1_s[:], ecode1_d)
        ecode2_s = const.tile([WIN, ph2.C], F32)
        nc.sync.dma_start(ecode2_s[:], ecode2_d)
        invd_s = const.tile([WIN, NW], F32)
        nc.sync.dma_start(invd_s[:], invd_d)

        ones1 = const.tile([1, WIN], BF16)
        nc.vector.memset(ones1[:], 1.0)
        iota = const.tile([WIN, WIN], F32)
        nc.gpsimd.iota(
            iota[:],
            pattern=[[1, WIN]],
            base=0,
            channel_multiplier=0,
            allow_small_or_imprecise_dtypes=True,
        )

        hT = const.tile([H, NW * WIN], BF16)

        # z shards / halo-exchange buffers, viewed as bf16 row-pairs
        zshA = dram.tile([p.rows_a // 2, 2 * O], BF16)
        zshB = dram.tile([p.rows_b // 2, 2 * O], BF16)
        zfullA = dram.tile([p.pairs_a, 2 * O], BF16, addr_space="Shared")
        zfullB = dram.tile([p.pairs_b, 2 * O], BF16, addr_space="Shared")

        def gather_slab(pool, ph, g, r, src_ap, elem, tag, eidx_s):
            s, n = ph.slab_meta[g][r]
            if n == 0:
                return None, s
            t = pool.tile([WIN, ph.max_slab[r], elem], BF16, tag=tag)
            for gi, i, nb, cs in ph.by_slab[(g, r)]:
                nc.gpsimd.dma_gather(
                    out_ap=t[:, i : i + nb, :],
                    in_ap=src_ap,
                    idxs_ap=eidx_s[:, cs * 8 : (cs + nb) * 8],
                    num_idxs=nb * WIN,
                    num_idxs_reg=nb * WIN,
                    elem_size=elem,
                    queue_num=next_q(),
                    single_packet=(nb <= 8),
                )
            return t, s

        def build_eq(pool, ph, ecode_s, g):
            """Batched 0/1 one-hot for all windows of group g: [WIN, n_g, WIN]."""
            g0, g1 = ph.groups[g]
            n = ph.group_nw[g]
            c0 = int(ph.wc0[g0])
            eq = pool.tile([WIN, ph.max_group_nw, WIN], BF16, tag="eq")
            nc.vector.tensor_tensor(
                out=eq[:, :n, :],
                in0=ecode_s[:, c0 : c0 + n].to_broadcast([WIN, n, WIN]),
                in1=iota[:].rearrange("p f -> p () f").to_broadcast([WIN, n, WIN]),
                op=AOT.is_equal,
            )
            return eq, c0

        # ---------------- phase 1 ----------------
        with (
            tc.tile_pool(name="gix1", bufs=1) as gixp,
            tc.tile_pool(name="xg", bufs=GATHER_BUFS) as xgp,
            tc.tile_pool(name="oh1", bufs=2) as ohp,
            tc.tile_pool(name="srg", bufs=2) as srp,
            tc.tile_pool(name="aggn", bufs=2) as aggp,
            tc.tile_pool(name="psA", bufs=2, space="PSUM") as psA,
            tc.tile_pool(name="psH", bufs=2, space="PSUM") as psH,
            tc.tile_pool(name="psZ", bufs=2, space="PSUM") as psZ,
        ):
            eidx1_s = gixp.tile([WIN, ph1.C * 8], I16)
            nc.sync.dma_start(eidx1_s[:], eidx1_d)

            def load_sr(g):
                g0, g1 = groups[g]
                t = srp.tile([WIN, GROUP * WIN], F32, tag="sr")
                nc.scalar.dma_start(
                    t[:, : (g1 - g0) * WIN], sr_d[:, g0 * WIN : g1 * WIN]
                )
                return t

            eq_tiles = {0: build_eq(ohp, ph1, ecode1_s, 0)}
            sr_tiles = {0: load_sr(0)}

            for g, (g0, g1) in enumerate(groups):
                slabs = {}
                for r in range(ph1.nranges):
                    r0 = r * RSZ1
                    r1 = min(r0 + RSZ1, p.n_nodes)
                    t, s = gather_slab(
                        xgp, ph1, g, r, x_d[r0:r1, :], F, f"xg{r}", eidx1_s
                    )
                    if t is not None:
                        slabs[r] = (t, s)

                if g + 1 < ngroups:
                    eq_tiles[g + 1] = build_eq(ohp, ph1, ecode1_s, g + 1)
                    sr_tiles[g + 1] = load_sr(g + 1)
                eq, eq_c0 = eq_tiles.pop(g)
                srg = sr_tiles.pop(g)

                for w in range(g0, g1):
                    rows = p.rows_last if w == NW - 1 else WIN
                    wsl = slice(w * WIN, (w + 1) * WIN)
                    chunks = ph1.window_chunks[w]
                    wcol = int(ph1.wc0[w]) - eq_c0

                    agg = psA.tile([F, WIN], F32, tag="agg")
                    for jj, (r, gid) in enumerate(chunks):
                        t, s = slabs[r]
                        nc.tensor.matmul(
                            out=agg[:],
                            lhsT=t[:, gid - s, :],
                            rhs=eq[:, wcol + jj, :],
                            start=(jj == 0),
                            stop=(jj == len(chunks) - 1),
                        )

                    # aggn = (agg * s_in[dst]) in bf16 (PSUM -> SBUF)
                    aggn = aggp.tile([F, WIN], BF16, tag="aggn")
                    nc.vector.tensor_tensor(
                        out=aggn[:],
                        in0=agg[:],
                        in1=srg[:, (w - g0) * WIN : (w - g0 + 1) * WIN],
                        op=AOT.mult,
                    )

                    hpre = psH.tile([H, WIN], F32, tag="hpre")
                    nc.tensor.matmul(
                        out=hpre[:], lhsT=W1s[:], rhs=aggn[:], start=True, stop=True
                    )
                    nc.scalar.activation(hT[:, wsl], hpre[:], AFT.Relu, bias=b1s[:])

                    zp = psZ.tile([WIN, O], F32, tag="zp")
                    nc.tensor.matmul(
                        out=zp[:], lhsT=hT[:, wsl], rhs=Wns[:], start=True, stop=True
                    )
                    zt = aggp.tile([WIN, O], BF16, tag="zt")
                    nc.scalar.activation(zt[:], zp[:], AFT.Copy)
                    if w < HALF_W:
                        nc.sync.dma_start(
                            zshA[w * (WIN // 2) : w * (WIN // 2) + rows // 2, :],
                            zt[:rows, :],
                        )
                    else:
                        wb = w - HALF_W
                        nc.sync.dma_start(
                            zshB[wb * (WIN // 2) : wb * (WIN // 2) + rows // 2, :],
                            zt[:rows, :],
                        )
        # ---------------- halo exchange (split; A can fire while phase-1
        # MMs drain, B after the full shard is written) ----
        nc.gpsimd.collective_compute(
            "AllGather", AOT.bypass,
            replica_groups=[list(range(p.n_cores))],
            ins=[zshA.opt()], outs=[zfullA.opt()],
        )
        nc.gpsimd.collective_compute(
            "AllGather", AOT.bypass,
            replica_groups=[list(range(p.n_cores))],
            ins=[zshB.opt()], outs=[zfullB.opt()],
        )

        # ---------------- phase 2 ----------------
        with (
            tc.tile_pool(name="gix2", bufs=1) as gixp2,
            tc.tile_pool(name="zg", bufs=GATHER_BUFS) as zgp,
            tc.tile_pool(name="oh2", bufs=2) as ohp2,
            tc.tile_pool(name="nm", bufs=2) as nmp,
            tc.tile_pool(name="psN", bufs=2, space="PSUM") as psN,
            tc.tile_pool(name="psS", bufs=2, space="PSUM") as psS,
        ):
            eidx2_s = gixp2.tile([WIN, ph2.C * 8], I16)
            nc.sync.dma_start(eidx2_s[:], eidx2_d)

            eq_tiles = {0: build_eq(ohp2, ph2, ecode2_s, 0)}
            zsrc = [zfullA.opt(), zfullB.opt()]
            for g, (g0, g1) in enumerate(groups):
                slabs = {}
                for r in range(ph2.nranges):
                    t, s = gather_slab(
                        zgp, ph2, g, r, zsrc[r], 2 * O, f"zg{r}", eidx2_s
                    )
                    if t is not None:
                        slabs[r] = (t, s)

                if g + 1 < ngroups:
                    eq_tiles[g + 1] = build_eq(ohp2, ph2, ecode2_s, g + 1)
                eq, eq_c0 = eq_tiles.pop(g)

                for w in range(g0, g1):
                    rows = p.rows_last if w == NW - 1 else WIN
                    wsl = slice(w * WIN, (w + 1) * WIN)
                    chunks = ph2.window_chunks[w]
                    wcol = int(ph2.wc0[w]) - eq_c0

                    nm = psN.tile([WIN, O], F32, tag="nm")
                    for jj, (cl, gid) in enumerate(chunks):
                        r, par = cl >> 1, cl & 1
                        t, s = slabs[r]
                        nc.tensor.matmul(
                            out=nm[:],
                            lhsT=eq[:, wcol + jj, :],
                            rhs=t[:, gid - s, par * O : (par + 1) * O],
                            start=(jj == 0),
                            stop=(jj == len(chunks) - 1),
                        )

                    sb = psS.tile([WIN, O], F32, tag="sb")
                    nc.tensor.matmul(
                        out=sb[:], lhsT=ones1[:], rhs=b2s[:], start=True, stop=False
                    )
                    nc.tensor.matmul(
                        out=sb[:], lhsT=hT[:, wsl], rhs=Wss[:], start=False, stop=True
                    )

                    nms = nmp.tile([WIN, O], F32, tag="nms")
                    nc.vector.tensor_scalar(
                        out=nms[:], in0=nm[:], scalar1=invd_s[:, w : w + 1],
                        scalar2=None, op0=AOT.mult,
                    )
                    outt = nmp.tile([WIN, O], F32, tag="outt")
                    nc.vector.tensor_tensor(outt[:], nms[:], sb[:], op=AOT.add)
                    nc.sync.dma_start(
                        out_d[w * WIN : w * WIN + rows, :], outt[:rows, :]
                    )

    nc.compile()
    return nc


# ---------------------------------------------------------------------------
# Entry point
# ---------------------------------------------------------------------------

_CACHE = {}


def _get_compiled(p, F, H, O):
    key = (p.n_nodes, p.n_cores, p.ph1.C, p.ph2.C, F, H, O)
    if key not in _CACHE:
        import time as _time

        t0 = _time.time()
        _CACHE[key] = build_gcn(p, F, H, O)
        if os.environ.get("GCN_VERBOSE"):
            print(f"[gcn] build+bass-compile: {_time.time() - t0:.1f}s", flush=True)
    return _CACHE[key]


def make_in_maps(p, inputs):
    H = np.asarray(inputs["W1"]).shape[1]
    O = np.asarray(inputs["W_self"]).shape[1]
    x = np.asarray(inputs["x"], np.float32)
    xb = (x * p.s_out[:, None]).astype(BF)
    base = dict(
        x=np.ascontiguousarray(xb),
        W1=np.ascontiguousarray(np.asarray(inputs["W1"], np.float32).astype(BF)),
        b1=np.ascontiguousarray(np.asarray(inputs["b1"], np.float32).reshape(H, 1)),
        W_self=np.ascontiguousarray(np.asarray(inputs["W_self"], np.float32).astype(BF)),
        W_neigh=np.ascontiguousarray(np.asarray(inputs["W_neigh"], np.float32).astype(BF)),
        b2=np.ascontiguousarray(np.asarray(inputs["b2"], np.float32).reshape(1, O).astype(BF)),
    )
    in_maps = []
    for k in range(p.n_cores):
        m = dict(base)
        m.update(p.per_core[k])
        in_maps.append(m)
    return in_maps


def kernel(**inputs):
    x = np.asarray(inputs["x"])
    src = np.asarray(inputs["src"])
    dst = np.asarray(inputs["dst"])
    n_nodes, F = x.shape
    H = np.asarray(inputs["W1"]).shape[1]
    O = np.asarray(inputs["W_self"]).shape[1]

    p = prepare(src, dst, n_nodes)
    nc = _get_compiled(p, F, H, O)
    in_maps = make_in_maps(p, inputs)
    res = run_bass_kernel_spmd(
        nc, in_maps, core_ids=list(range(p.n_cores)),
        trace=bool(int(os.environ.get("GCN_TRACE", "0"))),
    )
    if os.environ.get("GCN_RESULT_HOOK"):
        _CACHE["last_results"] = res
    out = np.concatenate([r["out"] for r in res.results], axis=0)
    return out.astype(np.float32)


# revision 17
# speedup vs baseline: 1.4011x; 1.0494x over previous
"""GCN (GraphConv norm='both' -> ReLU -> SAGEConv mean) on 8 Trainium2 NeuronCores.

Contract: kernel(**inputs) takes the FULL inputs from setup_inputs() and
returns the FULL [N, OUT] output.

Sharding strategy (graph/data parallel, per the problem's sharding hint):
  - Nodes are partitioned contiguously across the 8 cores (12500 each).
  - Edges are partitioned by the owner of their *dst* node; each core's
    edges are bucketed per 128-node dst window into padded 128-edge chunks
    and aggregated with one-hot matmuls on the TensorEngine.
  - All gathered/streamed feature data is bf16 (tolerance is 2e-2; the
    bf16 pipeline sims at ~4e-3), halving the dominant cost: the random
    256-byte-per-edge dma_gather traffic, and running the one-hot matmuls
    at full PE rate (fp32 matmul is 4 cyc/row, bf16 is 1).
  - The degree normalization is split so the one-hot matrices stay pure
    0/1 (one DVE is_equal per group instead of two ops): s_out[src] is
    folded into x on the host (xb = s_out*x in bf16), and s_in[dst] is a
    per-dst-column scale fused into the PSUM->SBUF copy before W1.
  - Phase 1 (per core): dma_gather xb rows (4 int16-index ranges),
    one-hot matmul segment-sum into PSUM, hT = relu(W1.T@(agg*s_in)+b1)
    kept SBUF-resident in bf16, z = h @ W_neigh written (bf16) to a local
    z shard viewed as row-pairs.
  - Halo exchange: z (bf16, 64 wide = 12.8 MB total) is AllGathered in
    TWO halves so the first collective overlaps the second half of
    phase 1.  Nodes are remapped (host-side) so each half is rank-major
    contiguous.
  - Phase 2 (per core): dma_gather z-row-PAIRS (256B descriptors -- the
    gather elem must be a multiple of 256B, so single 128B bf16 z rows
    cannot be gathered directly).  Host packs edges into parity-pure
    chunks so each chunk's matmul reads the correct 64-column half of the
    gathered pair.  Segment-sum with 0/1 one-hots, scale by 1/deg_in,
    add h @ W_self + b2, write the core's [12500, 64] fp32 output shard.
  - Host concatenates the 8 shards.

Engine assignment is chosen to avoid FIFO head-of-line serialization:
the full int16 gather-index arrays are preloaded into SBUF (no per-group
index DMAs on the Sync engine), all PSUM->SBUF copies run on the Scalar
engine, and the Vector engine runs only the one-hot builds (software-
pipelined one group ahead) plus small per-window scales.
"""

import os
import sys
from contextlib import ExitStack

import numpy as np
import ml_dtypes

BF = ml_dtypes.bfloat16

for _p in ("/opt/trn_rl_repo", "/opt/pypackages"):
    if _p not in sys.path:
        sys.path.append(_p)

import concourse.bacc as bacc
import concourse.bass as bass
import concourse.mybir as mybir
import concourse.tile as tile
from concourse.bass_utils import run_bass_kernel_spmd

F32 = mybir.dt.float32
BF16 = mybir.dt.bfloat16
I16 = mybir.dt.int16
AOT = mybir.AluOpType
AFT = mybir.ActivationFunctionType

N_CORES = 8
WIN = 128
RSZ1 = 32768          # phase-1 src index range (int16 limit)
GROUP = 4             # windows per gather/eq group
SUBCHUNKS = int(os.environ.get("GCN_SUB", "14"))   # max chunks per dma_gather
NQUEUES = 4
HALF_W = 49           # windows per allgather half (49*128 = 6272 rows)
GATHER_BUFS = int(os.environ.get("GCN_GB", "4"))
SCRATCH = int(os.environ.get("GCN_SCRATCH", "16384"))  # SWDGE desc carveout B/partition


def _install_ntff_hook_shim():
    """The agent image's antenv lacks axon_hooks; provide it so trace=True
    can capture NTFF profiles through libaxon."""
    try:
        from antenv import axon_hooks  # noqa: F401
        return
    except ImportError:
        pass
    try:
        import types

        import antenv
        from trn_agent_boot.trn_boot import _ntff_profile_via_ctypes

        mod = types.ModuleType("antenv.axon_hooks")
        mod._hook = _ntff_profile_via_ctypes("/opt/axon/libaxon_pjrt.so")

        def get_axon_ntff_profile_hook():
            return mod._hook

        def set_axon_ntff_profile_hook(h):
            mod._hook = h

        mod.get_axon_ntff_profile_hook = get_axon_ntff_profile_hook
        mod.set_axon_ntff_profile_hook = set_axon_ntff_profile_hook
        sys.modules["antenv.axon_hooks"] = mod
        antenv.axon_hooks = mod
    except Exception:
        pass


_install_ntff_hook_shim()


# ---------------------------------------------------------------------------
# Host-side graph prep
# ---------------------------------------------------------------------------

class Phase:
    """Chunked edge-bucket structure for one gather/segment-sum phase."""
    pass


def _build_phase(owner, wrow, code, idx_local, cell_of, ncells, range_of_cell,
                 n_cores, NW, sub):
    """Bucket edges into per-(window, cell) 128-slot chunks, padded to the
    max count over cores so the SPMD program is identical on all cores.

    cell_of: per-edge cell id in [0, ncells); range_of_cell: gather source
    range per cell (cells sharing a range share a gather slab).
    """
    nranges = int(max(range_of_cell)) + 1
    counts = np.zeros((n_cores, NW, ncells), np.int64)
    np.add.at(counts, (owner, wrow, cell_of), 1)
    cwr = (counts.max(axis=0) + WIN - 1) // WIN          # [NW, ncells]
    empty = cwr.sum(axis=1) == 0
    cwr[empty, 0] = 1
    n_w = cwr.sum(axis=1)

    groups = [(g0, min(g0 + GROUP, NW)) for g0 in range(0, NW, GROUP)]

    # gather order: group -> range -> window -> cell(in range) -> chunk
    cell_start = np.zeros((NW, ncells), np.int64)
    slab_meta = []                                       # [g][r] = (start, n)
    c = 0
    for g0, g1 in groups:
        metas = []
        for r in range(nranges):
            s = c
            for w in range(g0, g1):
                for cl in range(ncells):
                    if range_of_cell[cl] != r:
                        continue
                    cell_start[w, cl] = c
                    c += int(cwr[w, cl])
            metas.append((s, c - s))
        slab_meta.append(metas)
    C = c

    gathers = []                # (g, r, chunk_off_in_slab, nb, global_chunk)
    if os.environ.get("GCN_CELLGATHER", "0") == "1":
        # one gather per (window, cell): pad slots carry trailing -1 indices,
        # which the SWDGE gather ucode skips (no descriptors generated)
        for g, (g0, g1) in enumerate(groups):
            for r in range(nranges):
                s, n = slab_meta[g][r]
                for w in range(g0, g1):
                    for cl in range(ncells):
                        if range_of_cell[cl] != r:
                            continue
                        cs = int(cell_start[w, cl])
                        nb = int(cwr[w, cl])
                        for i in range(0, nb, sub):
                            nbb = min(sub, nb - i)
                            gathers.append((g, r, cs - s + i, nbb, cs + i))
    else:
        for g in range(len(groups)):
            for r in range(nranges):
                s, n = slab_meta[g][r]
                for i in range(0, n, sub):
                    nb = min(sub, n - i)
                    gathers.append((g, r, i, nb, s + i))

    # window-major chunk columns (for the one-hot code arrays)
    wc0 = np.zeros(NW, np.int64)
    wc0[1:] = np.cumsum(n_w)[:-1]

    window_chunks = []          # [w] -> list of (cell, gather_chunk_id)
    for w in range(NW):
        lst = []
        for cl in range(ncells):
            for j in range(int(cwr[w, cl])):
                lst.append((cl, int(cell_start[w, cl]) + j))
        window_chunks.append(lst)

    per_core = []
    for k in range(n_cores):
        m = owner == k
        key = (wrow[m] * ncells + cell_of[m]).astype(np.int64)
        order = np.argsort(key, kind="stable")
        key = key[order]
        e_idx = idx_local[m][order]
        e_code = code[m][order]
        bounds = np.searchsorted(key, np.arange(NW * ncells + 1))

        cellgather = os.environ.get("GCN_CELLGATHER", "0") == "1"
        A_idx = np.zeros(C * WIN, np.int16)
        W_code = np.full(C * WIN, 255.0, np.float32)
        for w in range(NW):
            woff = 0
            for cl in range(ncells):
                a, b = bounds[w * ncells + cl], bounds[w * ncells + cl + 1]
                n = b - a
                gbase = int(cell_start[w, cl]) * WIN
                wbase = (int(wc0[w]) + woff) * WIN
                woff += int(cwr[w, cl])
                if n > 0:
                    A_idx[gbase : gbase + n] = e_idx[a:b].astype(np.int16)
                    W_code[wbase : wbase + n] = e_code[a:b]
                if cellgather:
                    # trailing pads generate no DMA descriptors; keep >=128
                    # valid slots (a full partition sweep) so every SDMA
                    # engine gets a descriptor and increments the sem
                    slots = int(cwr[w, cl]) * WIN
                    A_idx[gbase + max(n, 128) : gbase + slots] = -1

        eidx = np.ascontiguousarray(np.tile(A_idx.reshape(C * 8, 16).T, (8, 1)))
        ecode = np.ascontiguousarray(W_code.reshape(C, WIN).T)
        per_core.append((eidx, ecode))

    ph = Phase()
    ph.nranges = nranges
    ph.cwr = cwr
    ph.n_w = n_w
    ph.wc0 = wc0
    ph.C = C
    ph.groups = groups
    ph.slab_meta = slab_meta
    ph.gathers = gathers
    ph.window_chunks = window_chunks
    ph.per_core = per_core
    ph.max_nw = int(n_w.max())
    ph.group_nw = [int(n_w[g0:g1].sum()) for g0, g1 in groups]
    ph.max_group_nw = max(ph.group_nw)
    ph.group_c0 = [slab_meta[g][0][0] for g in range(len(groups))]
    ph.group_c1 = ph.group_c0[1:] + [C]
    ph.max_slab = [
        max((slab_meta[g][r][1] for g in range(len(groups))), default=0)
        for r in range(nranges)
    ]
    by_slab = {}
    for gi, (g, r, i, nb, cs) in enumerate(gathers):
        by_slab.setdefault((g, r), []).append((gi, i, nb, cs))
    ph.by_slab = by_slab
    return ph


class Prep:
    pass


def prepare(src, dst, n_nodes, n_cores=N_CORES):
    src = np.asarray(src).astype(np.int64)
    dst = np.asarray(dst).astype(np.int64)
    P = n_nodes // n_cores
    assert P * n_cores == n_nodes
    NW = (P + WIN - 1) // WIN
    rows_last = P - WIN * (NW - 1)
    rows_a = HALF_W * WIN                # 6272 local rows in allgather half A
    rows_b = P - rows_a
    assert rows_a % 2 == 0 and rows_b % 2 == 0

    deg_out = np.bincount(src, minlength=n_nodes).astype(np.float32)
    deg_in = np.bincount(dst, minlength=n_nodes).astype(np.float32)
    s_out = np.where(deg_out > 0, 1.0 / np.sqrt(np.maximum(deg_out, 1.0)), 0.0)
    s_in = np.where(deg_in > 0, 1.0 / np.sqrt(np.maximum(deg_in, 1.0)), 0.0)
    invd = (1.0 / np.maximum(deg_in, 1.0)).astype(np.float32)

    owner = dst // P
    ldst = dst - owner * P
    wrow = ldst // WIN
    code = (ldst % WIN).astype(np.float32)

    # ---- phase 1: gather xb rows; cells = 4 int16 src ranges ----
    rng1 = np.minimum(src // RSZ1, 3)
    idx1 = src - rng1 * RSZ1
    ph1 = _build_phase(owner, wrow, code, idx1, rng1, 4, [0, 1, 2, 3],
                       n_cores, NW, SUBCHUNKS)

    # ---- phase 2: gather z row-pairs from the remapped (split-allgather)
    # z layout; cells = (pair-range A/B) x (row parity) ----
    sc = src // P
    sl = src - sc * P
    new_row = np.where(sl < rows_a,
                       sc * rows_a + sl,
                       n_cores * rows_a + sc * rows_b + (sl - rows_a))
    pairs_a = n_cores * rows_a // 2      # 25088
    pr = new_row >> 1
    parity = (new_row & 1).astype(np.int64)
    r2 = (pr >= pairs_a).astype(np.int64)
    idx2 = pr - r2 * pairs_a
    cell2 = r2 * 2 + parity
    ph2 = _build_phase(owner, wrow, code, idx2, cell2, 4, [0, 0, 1, 1],
                       n_cores, NW, SUBCHUNKS)

    per_core = []
    for k in range(n_cores):
        eidx1, ecode1 = ph1.per_core[k]
        eidx2, ecode2 = ph2.per_core[k]
        nodes = np.arange(P) + k * P
        iv = np.zeros(NW * WIN, np.float32)
        iv[:P] = invd[nodes]
        sr = np.zeros(NW * WIN, np.float32)
        sr[:P] = s_in[nodes]
        per_core.append(dict(
            eidx1=eidx1, ecode1=ecode1,
            eidx2=eidx2, ecode2=ecode2,
            invd=np.ascontiguousarray(iv.reshape(NW, WIN).T),
            sr=np.ascontiguousarray(np.broadcast_to(sr, (WIN, NW * WIN))),
        ))

    p = Prep()
    p.P, p.NW, p.rows_last = P, NW, rows_last
    p.rows_a, p.rows_b = rows_a, rows_b
    p.pairs_a = pairs_a
    p.pairs_b = (n_nodes - n_cores * rows_a) // 2
    p.ph1, p.ph2 = ph1, ph2
    p.per_core = per_core
    p.s_out = s_out
    p.n_nodes = n_nodes
    p.n_cores = n_cores
    return p


# ---------------------------------------------------------------------------
# Bass/Tile kernel builder
# ---------------------------------------------------------------------------

def build_gcn(p, F, H, O):
    NW, P = p.NW, p.P
    ph1, ph2 = p.ph1, p.ph2
    groups = ph1.groups
    ngroups = len(groups)

    nc = bacc.Bacc(
        "TRN2", debug=False, enable_asserts=False, num_devices=p.n_cores,
        num_swdge_queues=NQUEUES, dynamic_dma_scratch_size=SCRATCH,
    )

    x_d = nc.dram_tensor("x", [p.n_nodes, F], BF16, kind="ExternalInput").ap()
    W1_d = nc.dram_tensor("W1", [F, H], BF16, kind="ExternalInput").ap()
    b1_d = nc.dram_tensor("b1", [H, 1], F32, kind="ExternalInput").ap()
    Ws_d = nc.dram_tensor("W_self", [H, O], BF16, kind="ExternalInput").ap()
    Wn_d = nc.dram_tensor("W_neigh", [H, O], BF16, kind="ExternalInput").ap()
    b2_d = nc.dram_tensor("b2", [1, O], BF16, kind="ExternalInput").ap()
    eidx1_d = nc.dram_tensor("eidx1", [WIN, ph1.C * 8], I16, kind="ExternalInput").ap()
    ecode1_d = nc.dram_tensor("ecode1", [WIN, ph1.C], F32, kind="ExternalInput").ap()
    eidx2_d = nc.dram_tensor("eidx2", [WIN, ph2.C * 8], I16, kind="ExternalInput").ap()
    ecode2_d = nc.dram_tensor("ecode2", [WIN, ph2.C], F32, kind="ExternalInput").ap()
    invd_d = nc.dram_tensor("invd", [WIN, NW], F32, kind="ExternalInput").ap()
    sr_d = nc.dram_tensor("sr", [WIN, NW * WIN], F32, kind="ExternalInput").ap()
    out_d = nc.dram_tensor("out", [P, O], F32, kind="ExternalOutput").ap()

    qn = [0]

    def next_q():
        q = qn[0]
        qn[0] = (q + 1) % NQUEUES
        return q

    with tile.TileContext(nc, num_cores=p.n_cores) as tc, ExitStack() as ctx:
        const = ctx.enter_context(tc.tile_pool(name="const", bufs=1))
        dram = ctx.enter_context(tc.tile_pool(name="dram", bufs=1, space="DRAM"))

        W1s = const.tile([F, H], BF16)
        nc.sync.dma_start(W1s[:], W1_d)
        Wss = const.tile([H, O], BF16)
        nc.sync.dma_start(Wss[:], Ws_d)
        Wns = const.tile([H, O], BF16)
        nc.sync.dma_start(Wns[:], Wn_d)
        b1s = const.tile([H, 1], F32)
        nc.sync.dma_start(b1s[:], b1_d)
        b2s = const.tile([1, O], BF16)
        nc.sync.dma_start(b2s[:], b2_d)
        ecode1_s = const.tile([WIN, ph1.C], F32)
        nc.sync.dma_start(ecode1_s[:], ecode1_d)
        ecode2_s = const.tile([WIN, ph2.C], F32)
        nc.sync.dma_start(ecode2_s[:], ecode2_d)
        invd_s = const.tile([WIN, NW], F32)
        nc.sync.dma_start(invd_s[:], invd_d)

        ones1 = const.tile([1, WIN], BF16)
        nc.vector.memset(ones1[:], 1.0)
        iota = const.tile([WIN, WIN], F32)
        nc.gpsimd.iota(
            iota[:],
            pattern=[[1, WIN]],
            base=0,
            channel_multiplier=0,
            allow_small_or_imprecise_dtypes=True,
        )

        hT = const.tile([H, NW * WIN], BF16)

        # z shards / halo-exchange buffers, viewed as bf16 row-pairs
        zshA = dram.tile([p.rows_a // 2, 2 * O], BF16)
        zshB = dram.tile([p.rows_b // 2, 2 * O], BF16)
        zfullA = dram.tile([p.pairs_a, 2 * O], BF16, addr_space="Shared")
        zfullB = dram.tile([p.pairs_b, 2 * O], BF16, addr_space="Shared")

        def gather_group(pool, ph, g, src_aps, elem, tagp, eidx_s):
            """Allocate the group's slabs and emit their sub-gathers
            interleaved round-robin across ranges, so the first NQUEUES
            gathers hit distinct queues AND distinct slabs (max SWDGE
            queue concurrency at each group start)."""
            slabs = {}
            queues = []
            for r in range(ph.nranges):
                s, n = ph.slab_meta[g][r]
                if n == 0:
                    continue
                t = pool.tile([WIN, ph.max_slab[r], elem], BF16, tag=f"{tagp}{r}")
                slabs[r] = (t, s)
                queues.append([(r, gi, i, nb, cs)
                               for gi, i, nb, cs in ph.by_slab[(g, r)]])
            k = 0
            while any(queues):
                lst = queues[k % len(queues)]
                k += 1
                if not lst:
                    continue
                r, gi, i, nb, cs = lst.pop(0)
                t, s = slabs[r]
                nc.gpsimd.dma_gather(
                    out_ap=t[:, i : i + nb, :],
                    in_ap=src_aps[r],
                    idxs_ap=eidx_s[:, cs * 8 : (cs + nb) * 8],
                    num_idxs=nb * WIN,
                    num_idxs_reg=nb * WIN,
                    elem_size=elem,
                    queue_num=next_q(),
                    single_packet=(nb <= 8),
                )
            return slabs

        def build_eq(pool, ph, ecode_s, g):
            """Batched 0/1 one-hot for all windows of group g: [WIN, n_g, WIN]."""
            g0, g1 = ph.groups[g]
            n = ph.group_nw[g]
            c0 = int(ph.wc0[g0])
            eq = pool.tile([WIN, ph.max_group_nw, WIN], BF16, tag="eq")
            nc.vector.tensor_tensor(
                out=eq[:, :n, :],
                in0=ecode_s[:, c0 : c0 + n].to_broadcast([WIN, n, WIN]),
                in1=iota[:].rearrange("p f -> p () f").to_broadcast([WIN, n, WIN]),
                op=AOT.is_equal,
            )
            return eq, c0

        # ---------------- phase 1 ----------------
        with (
            tc.tile_pool(name="gix1", bufs=1) as gixp,
            tc.tile_pool(name="xg", bufs=GATHER_BUFS) as xgp,
            tc.tile_pool(name="oh1", bufs=2) as ohp,
            tc.tile_pool(name="srg", bufs=2) as srp,
            tc.tile_pool(name="aggn", bufs=2) as aggp,
            tc.tile_pool(name="psA", bufs=2, space="PSUM") as psA,
            tc.tile_pool(name="psH", bufs=2, space="PSUM") as psH,
            tc.tile_pool(name="psZ", bufs=2, space="PSUM") as psZ,
        ):
            eidx1_s = gixp.tile([WIN, ph1.C * 8], I16)
            nc.sync.dma_start(eidx1_s[:], eidx1_d)

            def load_sr(g):
                g0, g1 = groups[g]
                t = srp.tile([WIN, GROUP * WIN], F32, tag="sr")
                nc.scalar.dma_start(
                    t[:, : (g1 - g0) * WIN], sr_d[:, g0 * WIN : g1 * WIN]
                )
                return t

            eq_tiles = {0: build_eq(ohp, ph1, ecode1_s, 0)}
            sr_tiles = {0: load_sr(0)}

            xsrc = [
                x_d[r * RSZ1 : min(r * RSZ1 + RSZ1, p.n_nodes), :]
                for r in range(ph1.nranges)
            ]
            for g, (g0, g1) in enumerate(groups):
                slabs = gather_group(xgp, ph1, g, xsrc, F, "xg", eidx1_s)

                if g + 1 < ngroups:
                    eq_tiles[g + 1] = build_eq(ohp, ph1, ecode1_s, g + 1)
                    sr_tiles[g + 1] = load_sr(g + 1)
                eq, eq_c0 = eq_tiles.pop(g)
                srg = sr_tiles.pop(g)

                for w in range(g0, g1):
                    rows = p.rows_last if w == NW - 1 else WIN
                    wsl = slice(w * WIN, (w + 1) * WIN)
                    chunks = ph1.window_chunks[w]
                    wcol = int(ph1.wc0[w]) - eq_c0

                    agg = psA.tile([F, WIN], F32, tag="agg")
                    for jj, (r, gid) in enumerate(chunks):
                        t, s = slabs[r]
                        nc.tensor.matmul(
                            out=agg[:],
                            lhsT=t[:, gid - s, :],
                            rhs=eq[:, wcol + jj, :],
                            start=(jj == 0),
                            stop=(jj == len(chunks) - 1),
                        )

                    # aggn = (agg * s_in[dst]) in bf16 (PSUM -> SBUF)
                    aggn = aggp.tile([F, WIN], BF16, tag="aggn")
                    nc.vector.tensor_tensor(
                        out=aggn[:],
                        in0=agg[:],
                        in1=srg[:, (w - g0) * WIN : (w - g0 + 1) * WIN],
                        op=AOT.mult,
                    )

                    hpre = psH.tile([H, WIN], F32, tag="hpre")
                    nc.tensor.matmul(
                        out=hpre[:], lhsT=W1s[:], rhs=aggn[:], start=True, stop=True
                    )
                    nc.scalar.activation(hT[:, wsl], hpre[:], AFT.Relu, bias=b1s[:])

                    zp = psZ.tile([WIN, O], F32, tag="zp")
                    nc.tensor.matmul(
                        out=zp[:], lhsT=hT[:, wsl], rhs=Wns[:], start=True, stop=True
                    )
                    zt = aggp.tile([WIN, O], BF16, tag="zt")
                    nc.scalar.activation(zt[:], zp[:], AFT.Copy)
                    if w < HALF_W:
                        nc.sync.dma_start(
                            zshA[w * (WIN // 2) : w * (WIN // 2) + rows // 2, :],
                            zt[:rows, :],
                        )
                    else:
                        wb = w - HALF_W
                        nc.sync.dma_start(
                            zshB[wb * (WIN // 2) : wb * (WIN // 2) + rows // 2, :],
                            zt[:rows, :],
                        )
        # ---------------- halo exchange (split; A can fire while phase-1
        # MMs drain, B after the full shard is written) ----
        nc.gpsimd.collective_compute(
            "AllGather", AOT.bypass,
            replica_groups=[list(range(p.n_cores))],
            ins=[zshA.opt()], outs=[zfullA.opt()],
        )
        nc.gpsimd.collective_compute(
            "AllGather", AOT.bypass,
            replica_groups=[list(range(p.n_cores))],
            ins=[zshB.opt()], outs=[zfullB.opt()],
        )

        # ---------------- phase 2 ----------------
        with (
            tc.tile_pool(name="gix2", bufs=1) as gixp2,
            tc.tile_pool(name="zg", bufs=GATHER_BUFS) as zgp,
            tc.tile_pool(name="oh2", bufs=2) as ohp2,
            tc.tile_pool(name="nm", bufs=2) as nmp,
            tc.tile_pool(name="psN", bufs=2, space="PSUM") as psN,
            tc.tile_pool(name="psS", bufs=2, space="PSUM") as psS,
        ):
            eidx2_s = gixp2.tile([WIN, ph2.C * 8], I16)
            nc.sync.dma_start(eidx2_s[:], eidx2_d)

            eq_tiles = {0: build_eq(ohp2, ph2, ecode2_s, 0)}
            zsrc = [zfullA.opt(), zfullB.opt()]
            for g, (g0, g1) in enumerate(groups):
                slabs = gather_group(zgp, ph2, g, zsrc, 2 * O, "zg", eidx2_s)

                if g + 1 < ngroups:
                    eq_tiles[g + 1] = build_eq(ohp2, ph2, ecode2_s, g + 1)
                eq, eq_c0 = eq_tiles.pop(g)

                for w in range(g0, g1):
                    rows = p.rows_last if w == NW - 1 else WIN
                    wsl = slice(w * WIN, (w + 1) * WIN)
                    chunks = ph2.window_chunks[w]
                    wcol = int(ph2.wc0[w]) - eq_c0

                    nm = psN.tile([WIN, O], F32, tag="nm")
                    for jj, (cl, gid) in enumerate(chunks):
                        r, par = cl >> 1, cl & 1
                        t, s = slabs[r]
                        nc.tensor.matmul(
                            out=nm[:],
                            lhsT=eq[:, wcol + jj, :],
                            rhs=t[:, gid - s, par * O : (par + 1) * O],
                            start=(jj == 0),
                            stop=(jj == len(chunks) - 1),
                        )

                    sb = psS.tile([WIN, O], F32, tag="sb")
                    nc.tensor.matmul(
                        out=sb[:], lhsT=ones1[:], rhs=b2s[:], start=True, stop=False
                    )
                    nc.tensor.matmul(
                        out=sb[:], lhsT=hT[:, wsl], rhs=Wss[:], start=False, stop=True
                    )

                    nms = nmp.tile([WIN, O], F32, tag="nms")
                    nc.vector.tensor_scalar(
                        out=nms[:], in0=nm[:], scalar1=invd_s[:, w : w + 1],
                        scalar2=None, op0=AOT.mult,
                    )
                    outt = nmp.tile([WIN, O], F32, tag="outt")
                    nc.vector.tensor_tensor(outt[:], nms[:], sb[:], op=AOT.add)
                    nc.sync.dma_start(
                        out_d[w * WIN : w * WIN + rows, :], outt[:rows, :]
                    )

    nc.compile()
    return nc


# ---------------------------------------------------------------------------
# Entry point
# ---------------------------------------------------------------------------

_CACHE = {}


def _get_compiled(p, F, H, O):
    key = (p.n_nodes, p.n_cores, p.ph1.C, p.ph2.C, F, H, O)
    if key not in _CACHE:
        import time as _time

        t0 = _time.time()
        _CACHE[key] = build_gcn(p, F, H, O)
        if os.environ.get("GCN_VERBOSE"):
            print(f"[gcn] build+bass-compile: {_time.time() - t0:.1f}s", flush=True)
    return _CACHE[key]


def make_in_maps(p, inputs):
    H = np.asarray(inputs["W1"]).shape[1]
    O = np.asarray(inputs["W_self"]).shape[1]
    x = np.asarray(inputs["x"], np.float32)
    xb = (x * p.s_out[:, None]).astype(BF)
    base = dict(
        x=np.ascontiguousarray(xb),
        W1=np.ascontiguousarray(np.asarray(inputs["W1"], np.float32).astype(BF)),
        b1=np.ascontiguousarray(np.asarray(inputs["b1"], np.float32).reshape(H, 1)),
        W_self=np.ascontiguousarray(np.asarray(inputs["W_self"], np.float32).astype(BF)),
        W_neigh=np.ascontiguousarray(np.asarray(inputs["W_neigh"], np.float32).astype(BF)),
        b2=np.ascontiguousarray(np.asarray(inputs["b2"], np.float32).reshape(1, O).astype(BF)),
    )
    in_maps = []
    for k in range(p.n_cores):
        m = dict(base)
        m.update(p.per_core[k])
        in_maps.append(m)
    return in_maps


def kernel(**inputs):
    x = np.asarray(inputs["x"])
    src = np.asarray(inputs["src"])
    dst = np.asarray(inputs["dst"])
    n_nodes, F = x.shape
    H = np.asarray(inputs["W1"]).shape[1]
    O = np.asarray(inputs["W_self"]).shape[1]

    p = prepare(src, dst, n_nodes)
    nc = _get_compiled(p, F, H, O)
    in_maps = make_in_maps(p, inputs)
    res = run_bass_kernel_spmd(
        nc, in_maps, core_ids=list(range(p.n_cores)),
        trace=bool(int(os.environ.get("GCN_TRACE", "0"))),
    )
    if os.environ.get("GCN_RESULT_HOOK"):
        _CACHE["last_results"] = res
    out = np.concatenate([r["out"] for r in res.results], axis=0)
    return out.astype(np.float32)
